# revision 2
# baseline (speedup 1.0000x reference)
"""Sliding-window causal GQA attention (RoPE) for Trainium2, 8-core SPMD.

Problem: x:(4,2048,2048), Wq:(2048,2048), Wk/Wv:(512,2048), Wo:(2048,2048)
  q = rope(x @ Wq.T) 16 heads, k/v = (x @ Wk.T / x @ Wv.T) 4 kv heads (GQA x4),
  causal sliding-window attention (W=1024), out = z @ Wo.T.

Sharding: 8 cores = 4 batches x 2 head-groups (8 q-heads / 2 kv-heads each).
Each core computes a partial output (its head-group's Wo contribution) for its
batch; host sums the two partials per batch.

Per-core kernel (all matmuls f32r = full-rate FP22):
  - layout: qT/kT as (head_dim, L) ["transposed"], v as (L, head_dim)
  - scores computed transposed S.T (keys on partitions, queries free) so the
    softmax denominator comes from a ones-vector matmul (row form) and P.T
    feeds the PV matmul directly with no on-chip transposes.
  - no max-subtraction in softmax: logits are O(1) here, exp is safe.
  - sliding window at 128-block granularity: query-super of 256 x up to 10
    key-blocks; boundary blocks masked via precomputed 0/1 tiles.
  - inputs are host-prepacked so each DMA moves long contiguous runs per
    partition (8-32KB), keeping DMA packet counts low.
"""

import math
import numpy as np

H = 16
D = 4
WINDOW = 1024
THETA = 10000.0
N, L, E = 4, 2048, 2048
P = 128
DH = E // H          # 128 head dim
NH = H // 2          # 8 q heads per core
NKV = 2              # kv heads per core
NB = L // P          # 16 key blocks
NKT = E // P         # 16 contraction tiles
SCALE = 1.0 / math.sqrt(DH)

_NC = None


def _kbs_for_super(t):
    """Key blocks overlapping the window of query super t (256 queries)."""
    return list(range(max(0, 2 * t - 8), 2 * t + 2))


def build_nc():
    from contextlib import ExitStack
    from concourse import bacc, tile, mybir

    F32 = mybir.dt.float32
    F32R = mybir.dt.float32r
    EXP = mybir.ActivationFunctionType.Exp

    SHUF_SWAP = [i ^ 1 for i in range(32)]

    nc = bacc.Bacc("TRN2", target_bir_lowering=False, debug=False)
    # prepacked inputs (see _pack_core_inputs for layouts)
    xq = nc.dram_tensor("xq", [4 * P, NKT * 512], F32R, kind="ExternalInput").ap()
    wqp = nc.dram_tensor("wqp", [NH * P, NKT * DH], F32R, kind="ExternalInput").ap()
    wkv = nc.dram_tensor("wkv", [P, NKT * 512], F32R, kind="ExternalInput").ap()
    woT = nc.dram_tensor("woT", [NH * DH, E], F32R, kind="ExternalInput").ap()
    cosT = nc.dram_tensor("cosT", [P, L], F32, kind="ExternalInput").ap()
    sinT = nc.dram_tensor("sinT", [P, L], F32, kind="ExternalInput").ap()
    masks = nc.dram_tensor("masks", [4 * P, 256], mybir.dt.bfloat16, kind="ExternalInput").ap()
    out = nc.dram_tensor("out", [L, E], F32, kind="ExternalOutput").ap()
    zspill = nc.dram_tensor("zspill", [NH * P, L], F32R).ap()

    with tile.TileContext(nc) as tc, ExitStack() as stk:
        const = stk.enter_context(tc.tile_pool(name="const", bufs=1))
        ones_f = const.tile([P, 1], F32, tag="ones_f")
        nc.vector.memset(ones_f[:], 1.0)
        onesrow_f = const.tile([1, P], F32, tag="onesrow_f")
        nc.vector.memset(onesrow_f[:], 1.0)
        ones = const.tile([P, 1], F32R, tag="ones")
        nc.vector.tensor_copy(ones[:], ones_f[:])
        onesrow = const.tile([1, P], F32R, tag="onesrow")
        nc.vector.tensor_copy(onesrow[:], onesrow_f[:])
        # mask kinds: 0=diagA (k<=q), 1=diagB (k<=q-128),
        #             2=farA (k>=q+1), 3=farB (k>=q-127)
        mk = [const.tile([P, 256], mybir.dt.bfloat16, tag=f"mk{i}", name=f"mk{i}") for i in range(4)]
        for i in range(4):
            nc.sync.dma_start(out=mk[i][:], in_=masks[i * P:(i + 1) * P, :])

        resid = stk.enter_context(tc.tile_pool(name="resid", bufs=1))
        kT = [resid.tile([P, L], F32R, tag=f"kT{i}", name=f"kT{i}") for i in range(NKV)]
        kvw = resid.tile([P, NKT * 512], F32R, tag="kvw")
        for dc in range(4):
            nc.sync.dma_start(out=kvw[:, dc * 2048:(dc + 1) * 2048],
                              in_=wkv[:, dc * 2048:(dc + 1) * 2048])
        vt = [[resid.tile([P, P], F32R, tag=f"v{i}_{b}", name=f"v{i}_{b}") for b in range(NB)]
              for i in range(NKV)]

        def rope_evict(dest, psum, cos_sl, sin_sl, tmp_pool, n):
            # dest = psum * cos + pairswap(psum) * sin   (sin pre-signed)
            tmp = tmp_pool.tile([P, 512], F32, tag="ropetmp", name="ropetmp")
            nc.vector.stream_shuffle(tmp[:, :n], psum, SHUF_SWAP)
            nc.vector.tensor_mul(tmp[:, :n], tmp[:, :n], sin_sl)
            nc.vector.tensor_mul(dest, psum, cos_sl)
            nc.vector.tensor_add(dest, dest, tmp[:, :n])

        osb = stk.enter_context(tc.tile_pool(name="osb", bufs=3))
        pp = stk.enter_context(tc.tile_pool(name="pp", bufs=2, space="PSUM"))
        psp = stk.enter_context(tc.tile_pool(name="ps", bufs=3, space="PSUM"))
        pzp = stk.enter_context(tc.tile_pool(name="pz", bufs=2, space="PSUM"))
        pbp = stk.enter_context(tc.tile_pool(name="pb", bufs=1, space="PSUM"))
        with tc.tile_pool(name="quarter", bufs=2) as qpool, \
             tc.tile_pool(name="wq", bufs=2) as wqpool, \
             tc.tile_pool(name="work", bufs=3) as work, \
             tc.tile_pool(name="qt", bufs=3) as qtpool, \
             tc.tile_pool(name="zev", bufs=3) as zevpool, \
             tc.tile_pool(name="rtmp", bufs=1) as rtmp:
            for qtr in range(4):
                c0 = 512 * qtr
                xt = qpool.tile([P, NKT * 512], F32R, tag="xt")
                cos_q = qpool.tile([P, 512], F32, tag="cos", bufs=1)
                sin_q = qpool.tile([P, 512], F32, tag="sin", bufs=1)
                for dc in range(4):
                    nc.sync.dma_start(
                        out=xt[:, dc * 2048:(dc + 1) * 2048],
                        in_=xq[qtr * P:(qtr + 1) * P, dc * 2048:(dc + 1) * 2048])
                nc.sync.dma_start(out=cos_q[:], in_=cosT[:, c0:c0 + 512])
                nc.sync.dma_start(out=sin_q[:], in_=sinT[:, c0:c0 + 512])

                def xtile(kt, a, b):
                    return xt[:, kt * 512 + a: kt * 512 + b]

                # K projection (+RoPE) for both kv heads
                for kv in range(NKV):
                    pk = pp.tile([P, 512], mybir.dt.float32, tag="pp")
                    for kt in range(NKT):
                        nc.tensor.matmul(
                            pk[:],
                            kvw[:, kt * 512 + kv * DH: kt * 512 + (kv + 1) * DH],
                            xtile(kt, 0, 512),
                            start=(kt == 0), stop=(kt == NKT - 1),
                        )
                    rope_evict(kT[kv][:, c0:c0 + 512], pk[:], cos_q[:], sin_q[:], rtmp, 512)

                # V projection (both kv heads at once, natural layout)
                for lb in range(4):
                    pv = pp.tile([P, 512], mybir.dt.float32, tag="pp")
                    for kt in range(NKT):
                        nc.tensor.matmul(
                            pv[:, :NKV * DH],
                            xtile(kt, lb * P, (lb + 1) * P),
                            kvw[:, kt * 512 + 256: kt * 512 + 512],
                            start=(kt == 0), stop=(kt == NKT - 1),
                        )
                    for kv in range(NKV):
                        nc.scalar.copy(vt[kv][4 * qtr + lb][:], pv[:, kv * DH:(kv + 1) * DH])

                # Q projection + attention, head-major
                for h in range(NH):
                    kv = h // (NH // NKV)
                    wq = wqpool.tile([P, NKT * DH], F32R, tag="wqh")
                    nc.sync.dma_start(out=wq[:], in_=wqp[h * P:(h + 1) * P, :])
                    pq = pp.tile([P, 512], mybir.dt.float32, tag="pp")
                    for kt in range(NKT):
                        nc.tensor.matmul(
                            pq[:],
                            wq[:, kt * DH:(kt + 1) * DH],
                            xtile(kt, 0, 512),
                            start=(kt == 0), stop=(kt == NKT - 1),
                        )
                    qth = qtpool.tile([P, 512], F32R, tag="qt")
                    rope_evict(qth[:], pq[:], cos_q[:], sin_q[:], rtmp, 512)
                    for s in range(2):
                        t = 2 * qtr + s
                        qt = qth[:, s * 256:(s + 1) * 256]

                        kbs = _kbs_for_super(t)
                        nkb = len(kbs)
                        pt = work.tile([P, 2560], F32R, tag="pt")
                        # scores (transposed: keys on partitions) in chunks of 2 kb
                        for ci in range(0, nkb, 2):
                            cn = min(2, nkb - ci)
                            ps = psp.tile([P, 512], mybir.dt.float32, tag="ps")
                            for i in range(cn):
                                kb = kbs[ci + i]
                                nc.tensor.matmul(
                                    ps[:, i * 256:(i + 1) * 256],
                                    kT[kv][:, kb * P:(kb + 1) * P],
                                    qt,
                                    start=True, stop=True,
                                )
                            nc.scalar.activation(
                                pt[:, ci * 256:(ci + cn) * 256],
                                ps[:, :cn * 256], EXP, scale=SCALE)
                        # window masks on boundary blocks
                        for i, kb in enumerate(kbs):
                            kind = None
                            if kb == 2 * t:
                                kind = 0
                            elif kb == 2 * t + 1:
                                kind = 1
                            elif kb == 2 * t - 8:
                                kind = 2
                            elif kb == 2 * t - 7:
                                kind = 3
                            if kind is not None:
                                sl = pt[:, i * 256:(i + 1) * 256]
                                nc.vector.tensor_mul(sl, sl, mk[kind][:])
                        # denominator (ones matmul) + PV, accumulated over kbs
                        pz = pzp.tile([P, 256], mybir.dt.float32, tag="pz")
                        su = pbp.tile([1, 256], mybir.dt.float32, tag="su")
                        for i, kb in enumerate(kbs):
                            st, sp = (i == 0), (i == nkb - 1)
                            nc.tensor.matmul(
                                su[:], ones[:],
                                pt[:, i * 256:(i + 1) * 256],
                                start=st, stop=sp)
                            nc.tensor.matmul(
                                pz[:], vt[kv][kb][:],
                                pt[:, i * 256:(i + 1) * 256],
                                start=st, stop=sp)
                        # normalize: bcast sums across partitions (K=1 matmul),
                        # full-lane approx reciprocal, multiply.
                        sus = qtpool.tile([1, 256], F32R, tag="sus")
                        nc.vector.tensor_copy(sus[:], su[:])
                        bcps = psp.tile([P, 512], mybir.dt.float32, tag="ps")
                        nc.tensor.matmul(bcps[:, 0:256], onesrow[:], sus[:],
                                         start=True, stop=True)
                        rec = zevpool.tile([P, 256], F32, tag="rec")
                        nc.vector.reciprocal_approx_fast(rec[:], bcps[:, 0:256])
                        zev = zevpool.tile([P, 256], F32R, tag="zev")
                        nc.vector.tensor_mul(zev[:], pz[:], rec[:])
                        nc.sync.dma_start(
                            out=zspill[h * P:(h + 1) * P, t * 256:(t + 1) * 256],
                            in_=zev[:])

        # Output projection: out[q,:] += sum_h zTn_h[:,q].T @ woT[h]
        with tc.tile_pool(name="wo", bufs=1) as wopool, \
             tc.tile_pool(name="zin", bufs=3) as zinpool:
            wo = [wopool.tile([P, E], F32R, tag=f"wo{h}", name=f"wo{h}") for h in range(NH)]
            for h in range(NH):
                nc.sync.dma_start(out=wo[h][:], in_=woT[h * P:(h + 1) * P, :])
            for qsb in range(4):
                zin = [zinpool.tile([P, 512], F32R, tag=f"zin{h}", name=f"zin{h}") for h in range(NH)]
                for h in range(NH):
                    nc.sync.dma_start(
                        out=zin[h][:],
                        in_=zspill[h * P:(h + 1) * P, qsb * 512:(qsb + 1) * 512])
                for ec in range(4):
                    for qb in range(4):
                        po = psp.tile([P, 512], mybir.dt.float32, tag="ps")
                        for h in range(NH):
                            nc.tensor.matmul(
                                po[:],
                                zin[h][:, qb * P:(qb + 1) * P],
                                wo[h][:, ec * 512:(ec + 1) * 512],
                                start=(h == 0), stop=(h == NH - 1),
                            )
                        ot = osb.tile([P, 512], F32, tag="ot")
                        nc.scalar.copy(ot[:], po[:])
                        nc.sync.dma_start(
                            out=out[qsb * 512 + qb * P: qsb * 512 + (qb + 1) * P,
                                    ec * 512:(ec + 1) * 512],
                            in_=ot[:])

    nc.compile()
    return nc


def _host_tables():
    freqs = 1.0 / (THETA ** (np.arange(0, DH - 1, 2, dtype=np.float64) / DH))
    ang = np.arange(L, dtype=np.float64)[:, None] * freqs[None, :]  # (L, 64)
    cos = np.cos(ang)
    sin = np.sin(ang)
    cosT = np.empty((P, L), np.float32)
    sinT = np.empty((P, L), np.float32)
    cosT[0::2, :] = cos.T
    cosT[1::2, :] = cos.T
    sinT[0::2, :] = -sin.T
    sinT[1::2, :] = sin.T
    return cosT, sinT


def _host_masks():
    k = np.arange(P)[:, None]
    q = np.arange(256)[None, :]
    import ml_dtypes
    m = np.stack([
        (k <= q), (k <= q - 128), (k >= q + 1), (k >= q - 127),
    ]).astype(ml_dtypes.bfloat16)
    return m.reshape(4 * P, 256)


def _pack_core_inputs(x, Wq, Wk, Wv, Wo, n, g):
    """Prepacked per-core inputs; long contiguous per-partition DMA runs."""
    xT = np.ascontiguousarray(x[n].T)                      # (E, L)
    # xq[qtr*128+p, kt*512+c] = xT[kt*128+p, qtr*512+c]
    xq = xT.reshape(NKT, P, 4, 512).transpose(2, 1, 0, 3).reshape(4 * P, NKT * 512)
    # wqp[h*128+p, kt*128+c] = Wq.T[kt*128+p, g*1024+h*128+c]
    wqT = Wq[g * 1024:(g + 1) * 1024, :].T                 # (E, 1024)
    wqp = wqT.reshape(NKT, P, NH, DH).transpose(2, 1, 0, 3).reshape(NH * P, NKT * DH)
    # wkv[p, kt*512+j]: j<256 -> Wk.T slice, j>=256 -> Wv.T slice
    wkT = Wk[g * 256:(g + 1) * 256, :].T.reshape(NKT, P, 256)
    wvT = Wv[g * 256:(g + 1) * 256, :].T.reshape(NKT, P, 256)
    wkvp = np.concatenate([wkT, wvT], axis=2)              # (kt, p, 512)
    wkvp = wkvp.transpose(1, 0, 2).reshape(P, NKT * 512)
    woT = Wo[:, g * 1024:(g + 1) * 1024].T                 # (1024, E)
    return {
        "xq": np.ascontiguousarray(xq),
        "wqp": np.ascontiguousarray(wqp),
        "wkv": np.ascontiguousarray(wkvp),
        "woT": np.ascontiguousarray(woT),
    }


def _prepare_in_maps(x, Wq, Wk, Wv, Wo):
    cosT, sinT = _host_tables()
    masks = _host_masks()
    in_maps = []
    for c in range(8):
        n, g = c % 4, c // 4
        m = _pack_core_inputs(x, Wq, Wk, Wv, Wo, n, g)
        m.update({"cosT": cosT, "sinT": sinT, "masks": masks})
        in_maps.append(m)
    return in_maps


def kernel(x, Wq, Wk, Wv, Wo):
    global _NC
    x = np.asarray(x, np.float32)
    Wq = np.asarray(Wq, np.float32)
    Wk = np.asarray(Wk, np.float32)
    Wv = np.asarray(Wv, np.float32)
    Wo = np.asarray(Wo, np.float32)

    if _NC is None:
        _NC = build_nc()
    nc = _NC

    in_maps = _prepare_in_maps(x, Wq, Wk, Wv, Wo)

    from concourse.bass_utils import run_bass_kernel_spmd
    res = run_bass_kernel_spmd(nc, in_maps, list(range(8)), trace=False)
    out = np.empty((N, L, E), np.float32)
    for n_ in range(4):
        out[n_] = res.results[n_]["out"] + res.results[4 + n_]["out"]
    return out


if __name__ == "__main__":
    rng = np.random.default_rng(0)
    x = rng.standard_normal((N, L, E), dtype=np.float32)
    Wq = (rng.standard_normal((E, E), dtype=np.float32) * 0.02)
    Wk = (rng.standard_normal((E // D, E), dtype=np.float32) * 0.02)
    Wv = (rng.standard_normal((E // D, E), dtype=np.float32) * 0.02)
    Wo = (rng.standard_normal((E, E), dtype=np.float32) * 0.02)
    print(kernel(x, Wq, Wk, Wv, Wo).shape)



# revision 6
# speedup vs baseline: 1.0788x; 1.0788x over previous
"""Sliding-window causal GQA attention (RoPE) for Trainium2, 8-core SPMD.

Problem: x:(4,2048,2048), Wq:(2048,2048), Wk/Wv:(512,2048), Wo:(2048,2048)
  q = rope(x @ Wq.T) 16 heads, k/v = (x @ Wk.T / x @ Wv.T) 4 kv heads (GQA x4),
  causal sliding-window attention (W=1024), out = z @ Wo.T.

Sharding: 8 cores = 4 batches x 2 head-groups (8 q-heads / 2 kv-heads each).
Each core computes a partial output (its head-group's Wo contribution) for its
batch; host sums the two partials per batch.

Per-core kernel v2 (matmul operands bf16, f32 PSUM accumulation):
  - bf16 halves the PE's SBUF stream bytes plus all DMA/eviction traffic; the
    f32r baseline was SBUF-bandwidth-stretched (~1.3x matmul cadence).
  - head-PAIR batching: every scores/PV matmul covers two heads' 256-query
    supers -> free dim 512 everywhere.
  - softmax denominator: exp blocks tree-summed on the otherwise-idle Pool
    engine (split with DVE), then ONE all-ones matmul per pair broadcasts the
    column sums to all 128 partitions (no per-block ones-matmuls, no separate
    broadcast matmul).
  - Wo fused per query-super: z never leaves SBUF; Wo matmuls of the previous
    super are interleaved between attention matmuls as filler so the PE keeps
    streaming while the scalar engine drains exp evictions.
  - sliding window at 128-block granularity: query-super of 256 x up to 10
    key-blocks; boundary blocks masked via precomputed 0/1 tiles.
"""

import math
import numpy as np

H = 16
D = 4
WINDOW = 1024
THETA = 10000.0
N, L, E = 4, 2048, 2048
P = 128
DH = E // H          # 128 head dim
NH = H // 2          # 8 q heads per core
NKV = 2              # kv heads per core
NB = L // P          # 16 key blocks
NKT = E // P         # 16 contraction tiles
SCALE = 1.0 / math.sqrt(DH)

_NC = None


def _kbs_for_super(t):
    """Key blocks overlapping the window of query super t (256 queries)."""
    return list(range(max(0, 2 * t - 8), 2 * t + 2))


def build_nc():
    from contextlib import ExitStack
    from concourse import bacc, tile, mybir

    F32 = mybir.dt.float32
    F32R = mybir.dt.float32r
    BF16 = mybir.dt.bfloat16
    EXP = mybir.ActivationFunctionType.Exp

    SHUF_SWAP = [i ^ 1 for i in range(32)]

    nc = bacc.Bacc("TRN2", target_bir_lowering=False, debug=False)
    # prepacked inputs (see _pack_core_inputs for layouts)
    xq = nc.dram_tensor("xq", [4 * P, NKT * 512], BF16, kind="ExternalInput").ap()
    wqp = nc.dram_tensor("wqp", [NH * P, NKT * DH], BF16, kind="ExternalInput").ap()
    wkv = nc.dram_tensor("wkv", [P, NKT * 512], BF16, kind="ExternalInput").ap()
    woT = nc.dram_tensor("woT", [NH * DH, E], BF16, kind="ExternalInput").ap()
    cosT = nc.dram_tensor("cosT", [P, L], F32, kind="ExternalInput").ap()
    sinT = nc.dram_tensor("sinT", [P, L], F32, kind="ExternalInput").ap()
    masks = nc.dram_tensor("masks", [4 * P, 512], BF16, kind="ExternalInput").ap()
    out = nc.dram_tensor("out", [L, E], F32, kind="ExternalOutput").ap()

    with tile.TileContext(nc) as tc, ExitStack() as stk:
        const = stk.enter_context(tc.tile_pool(name="const", bufs=1))
        onesmat_f = const.tile([P, P], F32, tag="onesmat_f")
        nc.vector.memset(onesmat_f[:], 1.0)
        onesmat = const.tile([P, P], F32R, tag="onesmat")
        nc.vector.tensor_copy(onesmat[:], onesmat_f[:])
        # mask kinds (512 wide = two heads' 256-query supers):
        # 0=diagA (k<=q), 1=diagB (k<=q-128), 2=farA (k>=q+1), 3=farB (k>=q-127)
        mk = [const.tile([P, 512], BF16, tag=f"mk{i}", name=f"mk{i}") for i in range(4)]

        resid = stk.enter_context(tc.tile_pool(name="resid", bufs=1))
        kT = [resid.tile([P, L], BF16, tag=f"kT{i}", name=f"kT{i}") for i in range(NKV)]
        kvw = resid.tile([P, NKT * 512], BF16, tag="kvw")
        vt = [[resid.tile([P, P], BF16, tag=f"v{i}_{b}", name=f"v{i}_{b}") for b in range(NB)]
              for i in range(NKV)]
        wo = [resid.tile([P, E], BF16, tag=f"wo{h}", name=f"wo{h}") for h in range(NH)]

        def rope_evict(dest, psum, cos_sl, sin_sl, tmp_pool, n):
            # dest = psum * cos + pairswap(psum) * sin   (sin pre-signed)
            tmp = tmp_pool.tile([P, 512], F32, tag="ropetmp", name="ropetmp")
            nc.vector.stream_shuffle(tmp[:, :n], psum, SHUF_SWAP)
            nc.vector.tensor_mul(tmp[:, :n], tmp[:, :n], sin_sl)
            nc.vector.tensor_mul(dest, psum, cos_sl)
            nc.vector.tensor_add(dest, dest, tmp[:, :n])

        osb = stk.enter_context(tc.tile_pool(name="osb", bufs=3))
        pp = stk.enter_context(tc.tile_pool(name="pp", bufs=2, space="PSUM"))
        psp = stk.enter_context(tc.tile_pool(name="ps", bufs=3, space="PSUM"))
        pzp = stk.enter_context(tc.tile_pool(name="pz", bufs=2, space="PSUM"))
        pop = stk.enter_context(tc.tile_pool(name="po", bufs=1, space="PSUM"))
        with tc.tile_pool(name="quarter", bufs=2) as qpool, \
             tc.tile_pool(name="wq", bufs=2) as wqpool, \
             tc.tile_pool(name="work", bufs=22) as work, \
             tc.tile_pool(name="qt", bufs=2) as qtpool, \
             tc.tile_pool(name="accp", bufs=4) as accp, \
             tc.tile_pool(name="zsb", bufs=8) as zsbp, \
             tc.tile_pool(name="zev", bufs=4) as zevpool, \
             tc.tile_pool(name="rtmp", bufs=2) as rtmp:

            def wo_ops(zrows, t):
                """Generator emitting the fused-Wo matmuls for super t.

                Yields after each matmul so callers can interleave them as
                PE filler between attention matmuls.
                """
                for qb in range(2):
                    for ec in range(4):
                        po = pop.tile([P, 512], F32, tag="po")
                        for h in range(NH):
                            pr, hl = h // 2, h % 2
                            nc.tensor.matmul(
                                po[:],
                                zrows[pr][:, hl * 256 + qb * P:
                                          hl * 256 + (qb + 1) * P],
                                wo[h][:, ec * 512:(ec + 1) * 512],
                                start=(h == 0), stop=(h == NH - 1),
                            )
                            yield
                        ot = osb.tile([P, 512], F32, tag="ot")
                        nc.scalar.copy(ot[:], po[:])
                        r0 = t * 256 + qb * P
                        nc.sync.dma_start(
                            out=out[r0:r0 + P, ec * 512:(ec + 1) * 512],
                            in_=ot[:])

            # --- main loop ---
            prev_wo = iter(())   # filler generator for previous super's Wo
            for qtr in range(4):
                c0 = 512 * qtr
                xt = qpool.tile([P, NKT * 512], BF16, tag="xt")
                cos_q = qpool.tile([P, 512], F32, tag="cos", bufs=2)
                sin_q = qpool.tile([P, 512], F32, tag="sin", bufs=2)
                for dc in range(4):
                    nc.sync.dma_start(
                        out=xt[:, dc * 2048:(dc + 1) * 2048],
                        in_=xq[qtr * P:(qtr + 1) * P, dc * 2048:(dc + 1) * 2048])
                nc.sync.dma_start(out=cos_q[:], in_=cosT[:, c0:c0 + 512])
                nc.sync.dma_start(out=sin_q[:], in_=sinT[:, c0:c0 + 512])
                if qtr == 0:
                    for i in range(4):
                        nc.sync.dma_start(out=mk[i][:], in_=masks[i * P:(i + 1) * P, :])
                    for dc in range(4):
                        nc.sync.dma_start(out=kvw[:, dc * 2048:(dc + 1) * 2048],
                                          in_=wkv[:, dc * 2048:(dc + 1) * 2048])
                    for h in range(NH):
                        nc.sync.dma_start(out=wo[h][:], in_=woT[h * P:(h + 1) * P, :])

                def xtile(kt, a, b):
                    return xt[:, kt * 512 + a: kt * 512 + b]

                # K projection (+RoPE) for both kv heads
                for kv in range(NKV):
                    pk = pp.tile([P, 512], F32, tag="pp")
                    for kt in range(NKT):
                        nc.tensor.matmul(
                            pk[:],
                            kvw[:, kt * 512 + kv * DH: kt * 512 + (kv + 1) * DH],
                            xtile(kt, 0, 512),
                            start=(kt == 0), stop=(kt == NKT - 1),
                        )
                    rope_evict(kT[kv][:, c0:c0 + 512], pk[:], cos_q[:], sin_q[:], rtmp, 512)

                # V projection (both kv heads at once, natural layout)
                for lb in range(4):
                    pv = pp.tile([P, 512], F32, tag="pp")
                    for kt in range(NKT):
                        nc.tensor.matmul(
                            pv[:, :NKV * DH],
                            xtile(kt, lb * P, (lb + 1) * P),
                            kvw[:, kt * 512 + 256: kt * 512 + 512],
                            start=(kt == 0), stop=(kt == NKT - 1),
                        )
                    for kv in range(NKV):
                        nc.scalar.copy(vt[kv][4 * qtr + lb][:], pv[:, kv * DH:(kv + 1) * DH])

                # Q projection + RoPE into per-(super, kv) pair tiles
                qsup = [[qtpool.tile([P, 1024], BF16, tag=f"qs{s_}{g_}",
                                     name=f"qs{s_}{g_}")
                         for g_ in range(NKV)] for s_ in range(2)]
                for h in range(NH):
                    kv = h // (NH // NKV)
                    hl = h % (NH // NKV)
                    wq = wqpool.tile([P, NKT * DH], BF16, tag="wqh")
                    nc.sync.dma_start(out=wq[:], in_=wqp[h * P:(h + 1) * P, :])
                    pq = pp.tile([P, 512], F32, tag="pp")
                    for kt in range(NKT):
                        nc.tensor.matmul(
                            pq[:],
                            wq[:, kt * DH:(kt + 1) * DH],
                            xtile(kt, 0, 512),
                            start=(kt == 0), stop=(kt == NKT - 1),
                        )
                    for s_ in range(2):
                        rope_evict(
                            qsup[s_][kv][:, hl * 256:(hl + 1) * 256],
                            pq[:, s_ * 256:(s_ + 1) * 256],
                            cos_q[:, s_ * 256:(s_ + 1) * 256],
                            sin_q[:, s_ * 256:(s_ + 1) * 256],
                            rtmp, 256)

                # attention (+interleaved previous-super Wo) per super
                for s_ in range(2):
                    t = 2 * qtr + s_
                    kbs = _kbs_for_super(t)
                    nkb = len(kbs)
                    zrows = []   # 4 pair tiles: [dims, h_even 256q | h_odd 256q]
                    for kv in range(NKV):
                        qsv = qsup[s_][kv]
                        pt = [[None] * nkb for _ in range(2)]
                        accs = [None, None]
                        for p_ in range(2):
                            for ci in range(nkb):
                                kb = kbs[ci]
                                ps = psp.tile([P, 512], F32, tag="ps")
                                nc.tensor.matmul(
                                    ps[:],
                                    kT[kv][:, kb * P:(kb + 1) * P],
                                    qsv[:, p_ * 512:(p_ + 1) * 512],
                                    start=True, stop=True,
                                )
                                next(prev_wo, None)
                                ptt = work.tile([P, 512], BF16, tag="pt",
                                                name="ptt")
                                nc.scalar.activation(ptt[:], ps[:], EXP,
                                                     scale=SCALE)
                                kind = None
                                if kb == 2 * t:
                                    kind = 0
                                elif kb == 2 * t + 1:
                                    kind = 1
                                elif kb == 2 * t - 8:
                                    kind = 2
                                elif kb == 2 * t - 7:
                                    kind = 3
                                if kind is not None:
                                    nc.vector.tensor_mul(ptt[:], ptt[:],
                                                         mk[kind][:])
                                pt[p_][ci] = ptt
                            # denominator tree (DVE + Pool split)
                            acc = accp.tile([P, 512], F32R, tag="acc",
                                            name="acc")
                            eng = [nc.vector, nc.gpsimd]
                            if nkb >= 2:
                                eng[p_].tensor_add(acc[:], pt[p_][0][:],
                                                   pt[p_][1][:])
                            else:
                                eng[p_].tensor_copy(acc[:], pt[p_][0][:])
                            for ci in range(2, nkb):
                                eng[(ci + p_) % 2].tensor_add(acc[:], acc[:],
                                                              pt[p_][ci][:])
                            accs[p_] = acc

                        for p_ in range(2):
                            pz = pzp.tile([P, 512], F32, tag="pz")
                            for ci in range(nkb):
                                nc.tensor.matmul(
                                    pz[:], vt[kv][kbs[ci]][:], pt[p_][ci][:],
                                    start=(ci == 0), stop=(ci == nkb - 1))
                                if ci % 2 == 0:
                                    next(prev_wo, None)
                            # bcast denominator: all-ones stationary matmul
                            bc = psp.tile([P, 512], F32, tag="ps")
                            nc.tensor.matmul(bc[:], onesmat[:], accs[p_][:],
                                             start=True, stop=True)
                            rec = zevpool.tile([P, 512], F32, tag="rec")
                            nc.vector.reciprocal_approx_fast(rec[:], bc[:])
                            zev = zsbp.tile([P, 512], BF16, tag="zev")
                            nc.vector.tensor_mul(zev[:], pz[:], rec[:])
                            zrows.append(zev)
                    # drain leftover previous-super Wo, then arm this super's
                    for _ in prev_wo:
                        pass
                    prev_wo = wo_ops(zrows, t)
            for _ in prev_wo:
                pass

    nc.compile()
    return nc


def _host_tables():
    freqs = 1.0 / (THETA ** (np.arange(0, DH - 1, 2, dtype=np.float64) / DH))
    ang = np.arange(L, dtype=np.float64)[:, None] * freqs[None, :]  # (L, 64)
    cos = np.cos(ang)
    sin = np.sin(ang)
    cosT = np.empty((P, L), np.float32)
    sinT = np.empty((P, L), np.float32)
    cosT[0::2, :] = cos.T
    cosT[1::2, :] = cos.T
    sinT[0::2, :] = -sin.T
    sinT[1::2, :] = sin.T
    return cosT, sinT


def _host_masks():
    import ml_dtypes
    k = np.arange(P)[:, None]
    q = np.arange(256)[None, :]
    m = np.stack([
        (k <= q), (k <= q - 128), (k >= q + 1), (k >= q - 127),
    ]).astype(ml_dtypes.bfloat16)                     # (4, 128, 256)
    m2 = np.concatenate([m, m], axis=2)               # tile for head pairs
    return np.ascontiguousarray(m2.reshape(4 * P, 512))


def _pack_core_inputs(x, Wq, Wk, Wv, Wo, n, g):
    """Prepacked per-core inputs (bf16); long contiguous per-partition runs."""
    import ml_dtypes
    BF = ml_dtypes.bfloat16
    xT = np.ascontiguousarray(x[n].T)                      # (E, L)
    # xq[qtr*128+p, kt*512+c] = xT[kt*128+p, qtr*512+c]
    xq = xT.reshape(NKT, P, 4, 512).transpose(2, 1, 0, 3).reshape(4 * P, NKT * 512)
    # wqp[h*128+p, kt*128+c] = Wq.T[kt*128+p, g*1024+h*128+c]
    wqT = Wq[g * 1024:(g + 1) * 1024, :].T                 # (E, 1024)
    wqp = wqT.reshape(NKT, P, NH, DH).transpose(2, 1, 0, 3).reshape(NH * P, NKT * DH)
    # wkv[p, kt*512+j]: j<256 -> Wk.T slice, j>=256 -> Wv.T slice
    wkT = Wk[g * 256:(g + 1) * 256, :].T.reshape(NKT, P, 256)
    wvT = Wv[g * 256:(g + 1) * 256, :].T.reshape(NKT, P, 256)
    wkvp = np.concatenate([wkT, wvT], axis=2)              # (kt, p, 512)
    wkvp = wkvp.transpose(1, 0, 2).reshape(P, NKT * 512)
    woT = Wo[:, g * 1024:(g + 1) * 1024].T                 # (1024, E)
    return {
        "xq": np.ascontiguousarray(xq.astype(BF)),
        "wqp": np.ascontiguousarray(wqp.astype(BF)),
        "wkv": np.ascontiguousarray(wkvp.astype(BF)),
        "woT": np.ascontiguousarray(woT.astype(BF)),
    }


def _prepare_in_maps(x, Wq, Wk, Wv, Wo):
    cosT, sinT = _host_tables()
    masks = _host_masks()
    in_maps = []
    for c in range(8):
        n, g = c % 4, c // 4
        m = _pack_core_inputs(x, Wq, Wk, Wv, Wo, n, g)
        m.update({"cosT": cosT, "sinT": sinT, "masks": masks})
        in_maps.append(m)
    return in_maps


def kernel(x, Wq, Wk, Wv, Wo):
    global _NC
    x = np.asarray(x, np.float32)
    Wq = np.asarray(Wq, np.float32)
    Wk = np.asarray(Wk, np.float32)
    Wv = np.asarray(Wv, np.float32)
    Wo = np.asarray(Wo, np.float32)

    if _NC is None:
        _NC = build_nc()
    nc = _NC

    in_maps = _prepare_in_maps(x, Wq, Wk, Wv, Wo)

    from concourse.bass_utils import run_bass_kernel_spmd
    res = run_bass_kernel_spmd(nc, in_maps, list(range(8)), trace=False)
    out = np.empty((N, L, E), np.float32)
    for n_ in range(4):
        out[n_] = res.results[n_]["out"] + res.results[4 + n_]["out"]
    return out


if __name__ == "__main__":
    rng = np.random.default_rng(0)
    x = rng.standard_normal((N, L, E), dtype=np.float32)
    Wq = (rng.standard_normal((E, E), dtype=np.float32) * 0.02)
    Wk = (rng.standard_normal((E // D, E), dtype=np.float32) * 0.02)
    Wv = (rng.standard_normal((E // D, E), dtype=np.float32) * 0.02)
    Wo = (rng.standard_normal((E, E), dtype=np.float32) * 0.02)
    print(kernel(x, Wq, Wk, Wv, Wo).shape)


# revision 8
# speedup vs baseline: 1.4606x; 1.3540x over previous
"""Sliding-window causal GQA attention (RoPE) for Trainium2, 8-core SPMD.

Problem: x:(4,2048,2048), Wq:(2048,2048), Wk/Wv:(512,2048), Wo:(2048,2048)
  q = rope(x @ Wq.T) 16 heads, k/v = (x @ Wk.T / x @ Wv.T) 4 kv heads (GQA x4),
  causal sliding-window attention (W=1024), out = z @ Wo.T.

Sharding: 8 cores = 4 batches x 2 head-groups (8 q-heads / 2 kv-heads each).
Each core computes a partial output (its head-group's Wo contribution) for its
batch; host sums the two partials per batch.

Per-core kernel v3 (matmul operands bf16, f32 PSUM accumulation):
  - bf16 halves the PE's SBUF stream bytes plus all DMA/eviction traffic.
  - head-PAIR batching: every scores/PV matmul covers two heads' 256-query
    supers -> free dim 512 everywhere.  Q tiles are [128, 2, 512] per pair;
    the scores matmul gathers [2 heads x 256 q] via a 2-D free AP, so RoPE
    evicts full 512-wide per head (half the DVE instruction count).
  - softmax denominator: running per-pair accumulators on DVE (odd key
    blocks) and Pool (even), then all-ones-stationary matmuls accumulate the
    broadcast column sums directly into PSUM -> reciprocal -> scale.  Short
    critical path, no per-block ones-matmuls.
  - Wo fused per query-super: z never leaves SBUF; Wo matmuls of the previous
    super are interleaved between attention matmuls as PE filler while the
    scalar engine drains exp evictions.
  - DMA split across both hardware queues (SP: x/kv/wo stream; Act: wq + out
    writes) so projection weights never stall the PE.
"""

import math
import numpy as np

H = 16
D = 4
WINDOW = 1024
THETA = 10000.0
N, L, E = 4, 2048, 2048
P = 128
DH = E // H          # 128 head dim
NH = H // 2          # 8 q heads per core
NKV = 2              # kv heads per core
NB = L // P          # 16 key blocks
NKT = E // P         # 16 contraction tiles
SCALE = 1.0 / math.sqrt(DH)

_NC = None


def _kbs_for_super(t):
    """Key blocks overlapping the window of query super t (256 queries)."""
    return list(range(max(0, 2 * t - 8), 2 * t + 2))


def build_nc():
    from contextlib import ExitStack
    from concourse import bacc, tile, mybir

    F32 = mybir.dt.float32
    F32R = mybir.dt.float32r
    BF16 = mybir.dt.bfloat16
    EXP = mybir.ActivationFunctionType.Exp

    SHUF_SWAP = [i ^ 1 for i in range(32)]

    nc = bacc.Bacc("TRN2", target_bir_lowering=False, debug=False)
    # prepacked inputs (see _pack_core_inputs for layouts)
    xq = nc.dram_tensor("xq", [4 * P, NKT * 512], BF16, kind="ExternalInput").ap()
    wqp = nc.dram_tensor("wqp", [NH * P, NKT * DH], BF16, kind="ExternalInput").ap()
    wkv = nc.dram_tensor("wkv", [P, NKT * 512], BF16, kind="ExternalInput").ap()
    woT = nc.dram_tensor("woT", [NH * DH, E], BF16, kind="ExternalInput").ap()
    cosT = nc.dram_tensor("cosT", [P, L], F32, kind="ExternalInput").ap()
    sinT = nc.dram_tensor("sinT", [P, L], F32, kind="ExternalInput").ap()
    masks = nc.dram_tensor("masks", [4 * P, 512], BF16, kind="ExternalInput").ap()
    out = nc.dram_tensor("out", [L, E], F32, kind="ExternalOutput").ap()

    with tile.TileContext(nc) as tc, ExitStack() as stk:
        const = stk.enter_context(tc.tile_pool(name="const", bufs=1))
        onesmat_f = const.tile([P, P], F32, tag="onesmat_f")
        nc.vector.memset(onesmat_f[:], 1.0)
        onesmat = const.tile([P, P], F32R, tag="onesmat")
        nc.vector.tensor_copy(onesmat[:], onesmat_f[:])
        onesmat_b = const.tile([P, P], BF16, tag="onesmat_b")
        nc.vector.tensor_copy(onesmat_b[:], onesmat_f[:])
        # mask kinds (512 wide = two heads' 256-query supers):
        # 0=diagA (k<=q), 1=diagB (k<=q-128), 2=farA (k>=q+1), 3=farB (k>=q-127)
        mk = [const.tile([P, 512], BF16, tag=f"mk{i}", name=f"mk{i}") for i in range(4)]

        resid = stk.enter_context(tc.tile_pool(name="resid", bufs=1))
        kT = [resid.tile([P, L], BF16, tag=f"kT{i}", name=f"kT{i}") for i in range(NKV)]
        kvw = resid.tile([P, NKT * 512], BF16, tag="kvw")
        vt = [[resid.tile([P, P], BF16, tag=f"v{i}_{b}", name=f"v{i}_{b}") for b in range(NB)]
              for i in range(NKV)]
        wo = [resid.tile([P, E], BF16, tag=f"wo{h}", name=f"wo{h}") for h in range(NH)]

        def rope_evict(dest, psum, cos_sl, sin_sl, tmp_pool, n):
            # dest = psum * cos + pairswap(psum) * sin   (sin pre-signed)
            tmp = tmp_pool.tile([P, 512], F32, tag="ropetmp", name="ropetmp")
            nc.vector.stream_shuffle(tmp[:, :n], psum, SHUF_SWAP)
            nc.vector.tensor_mul(tmp[:, :n], tmp[:, :n], sin_sl)
            nc.vector.tensor_mul(dest, psum, cos_sl)
            nc.vector.tensor_add(dest, dest, tmp[:, :n])

        osb = stk.enter_context(tc.tile_pool(name="osb", bufs=2))
        pp = stk.enter_context(tc.tile_pool(name="pp", bufs=2, space="PSUM"))
        psp = stk.enter_context(tc.tile_pool(name="ps", bufs=3, space="PSUM"))
        pzp = stk.enter_context(tc.tile_pool(name="pz", bufs=2, space="PSUM"))
        pop = stk.enter_context(tc.tile_pool(name="po", bufs=1, space="PSUM"))
        with tc.tile_pool(name="quarter", bufs=2) as qpool, \
             tc.tile_pool(name="wq", bufs=3) as wqpool, \
             tc.tile_pool(name="work", bufs=21) as work, \
             tc.tile_pool(name="qt", bufs=2) as qtpool, \
             tc.tile_pool(name="accp", bufs=4) as accp, \
             tc.tile_pool(name="zsb", bufs=12) as zsbp, \
             tc.tile_pool(name="zev", bufs=3) as zevpool, \
             tc.tile_pool(name="rtmp", bufs=2) as rtmp:

            def wo_ops(zrows, t):
                """Generator emitting the fused-Wo matmuls for super t.

                Yields after each matmul so callers can interleave them as
                PE filler between attention matmuls.
                """
                for qb in range(2):
                    for ec in range(4):
                        po = pop.tile([P, 512], F32, tag="po")
                        for h in range(NH):
                            pr, hl = h // 2, h % 2
                            nc.tensor.matmul(
                                po[:],
                                zrows[pr][:, hl * 256 + qb * P:
                                          hl * 256 + (qb + 1) * P],
                                wo[h][:, ec * 512:(ec + 1) * 512],
                                start=(h == 0), stop=(h == NH - 1),
                            )
                            yield
                        ot = osb.tile([P, 512], F32, tag="ot")
                        nc.scalar.copy(ot[:], po[:])
                        r0 = t * 256 + qb * P
                        nc.sync.dma_start(
                            out=out[r0:r0 + P, ec * 512:(ec + 1) * 512],
                            in_=ot[:])

            # --- main loop ---
            prev_wo = iter(())   # filler generator for previous super's Wo
            for qtr in range(4):
                c0 = 512 * qtr
                xt = qpool.tile([P, NKT * 512], BF16, tag="xt")
                cos_q = qpool.tile([P, 512], F32, tag="cos", bufs=2)
                sin_q = qpool.tile([P, 512], F32, tag="sin", bufs=2)
                if qtr == 0:
                    # interleave kvw/x chunks so the K projection can start
                    # after the first pair lands; weights stream afterwards.
                    for dc in range(4):
                        nc.sync.dma_start(
                            out=kvw[:, dc * 2048:(dc + 1) * 2048],
                            in_=wkv[:, dc * 2048:(dc + 1) * 2048])
                        nc.sync.dma_start(
                            out=xt[:, dc * 2048:(dc + 1) * 2048],
                            in_=xq[qtr * P:(qtr + 1) * P, dc * 2048:(dc + 1) * 2048])
                else:
                    for dc in range(4):
                        nc.sync.dma_start(
                            out=xt[:, dc * 2048:(dc + 1) * 2048],
                            in_=xq[qtr * P:(qtr + 1) * P, dc * 2048:(dc + 1) * 2048])
                nc.sync.dma_start(out=cos_q[:], in_=cosT[:, c0:c0 + 512])
                nc.sync.dma_start(out=sin_q[:], in_=sinT[:, c0:c0 + 512])
                if qtr == 0:
                    for i in range(4):
                        nc.sync.dma_start(out=mk[i][:], in_=masks[i * P:(i + 1) * P, :])
                    for h in range(NH):
                        nc.sync.dma_start(out=wo[h][:], in_=woT[h * P:(h + 1) * P, :])

                def xtile(kt, a, b):
                    return xt[:, kt * 512 + a: kt * 512 + b]

                # K projection (+RoPE) for both kv heads
                for kv in range(NKV):
                    pk = pp.tile([P, 512], F32, tag="pp")
                    for kt in range(NKT):
                        nc.tensor.matmul(
                            pk[:],
                            kvw[:, kt * 512 + kv * DH: kt * 512 + (kv + 1) * DH],
                            xtile(kt, 0, 512),
                            start=(kt == 0), stop=(kt == NKT - 1),
                        )
                    rope_evict(kT[kv][:, c0:c0 + 512], pk[:], cos_q[:], sin_q[:], rtmp, 512)

                # V projection (both kv heads at once, natural layout)
                for lb in range(4):
                    pv = pp.tile([P, 512], F32, tag="pp")
                    for kt in range(NKT):
                        nc.tensor.matmul(
                            pv[:, :NKV * DH],
                            xtile(kt, lb * P, (lb + 1) * P),
                            kvw[:, kt * 512 + 256: kt * 512 + 512],
                            start=(kt == 0), stop=(kt == NKT - 1),
                        )
                    for kv in range(NKV):
                        nc.scalar.copy(vt[kv][4 * qtr + lb][:], pv[:, kv * DH:(kv + 1) * DH])

                # Q projection + RoPE into per-(kv, pair) [P, 2, 512] tiles
                qpair = [[qtpool.tile([P, 2, 512], BF16, tag=f"qp{g_}{pr_}",
                                      name=f"qp{g_}{pr_}")
                          for pr_ in range(2)] for g_ in range(NKV)]
                for h in range(NH):
                    kv, pr, hl = h // 4, (h // 2) % 2, h % 2
                    wq = wqpool.tile([P, NKT * DH], BF16, tag="wqh")
                    nc.sync.dma_start(out=wq[:], in_=wqp[h * P:(h + 1) * P, :])
                    pq = pp.tile([P, 512], F32, tag="pp")
                    for kt in range(NKT):
                        nc.tensor.matmul(
                            pq[:],
                            wq[:, kt * DH:(kt + 1) * DH],
                            xtile(kt, 0, 512),
                            start=(kt == 0), stop=(kt == NKT - 1),
                        )
                    rope_evict(qpair[kv][pr][:, hl, :], pq[:],
                               cos_q[:], sin_q[:], rtmp, 512)

                # attention (+interleaved previous-super Wo) per super
                for s_ in range(2):
                    t = 2 * qtr + s_
                    kbs = _kbs_for_super(t)
                    nkb = len(kbs)
                    odds = [ci for ci in range(nkb) if ci % 2 == 1]
                    evens = [ci for ci in range(nkb) if ci % 2 == 0]
                    zrows = []   # 4 pair tiles: [dims, h_even 256q | h_odd 256q]
                    for kv in range(NKV):
                        pt = [[None] * nkb for _ in range(2)]
                        accd = [None, None]   # DVE accumulators (odd blocks)
                        accg = [None, None]   # Pool accumulators (even blocks)
                        for p_ in range(2):
                            qmov = qpair[kv][p_][:, :, s_ * 256:(s_ + 1) * 256]
                            for ci in range(nkb):
                                kb = kbs[ci]
                                ps = psp.tile([P, 512], F32, tag="ps")
                                nc.tensor.matmul(
                                    ps[:],
                                    kT[kv][:, kb * P:(kb + 1) * P],
                                    qmov,
                                    start=True, stop=True,
                                )
                                next(prev_wo, None)
                                ptt = work.tile([P, 512], BF16, tag="pt",
                                                name="ptt")
                                nc.scalar.activation(ptt[:], ps[:], EXP,
                                                     scale=SCALE)
                                kind = None
                                if kb == 2 * t:
                                    kind = 0
                                elif kb == 2 * t + 1:
                                    kind = 1
                                elif kb == 2 * t - 8:
                                    kind = 2
                                elif kb == 2 * t - 7:
                                    kind = 3
                                if kind is not None:
                                    nc.vector.tensor_mul(ptt[:], ptt[:],
                                                         mk[kind][:])
                                pt[p_][ci] = ptt
                                # running denominator partial sums
                                if ci % 2 == 1 and len(odds) > 1:
                                    if ci == odds[1]:
                                        acc = accp.tile([P, 512], F32R,
                                                        tag="acc", name="acc")
                                        nc.vector.tensor_add(
                                            acc[:], pt[p_][odds[0]][:], ptt[:])
                                        accd[p_] = acc
                                    elif ci > odds[1]:
                                        nc.vector.tensor_add(
                                            accd[p_][:], accd[p_][:], ptt[:])
                                elif ci % 2 == 0 and len(evens) > 1:
                                    if ci == evens[1]:
                                        acc = accp.tile([P, 512], F32R,
                                                        tag="acc", name="acc")
                                        nc.gpsimd.tensor_add(
                                            acc[:], pt[p_][evens[0]][:], ptt[:])
                                        accg[p_] = acc
                                    elif ci > evens[1]:
                                        nc.gpsimd.tensor_add(
                                            accg[p_][:], accg[p_][:], ptt[:])

                        for p_ in range(2):
                            pz = pzp.tile([P, 512], F32, tag="pz")
                            for ci in range(nkb):
                                nc.tensor.matmul(
                                    pz[:], vt[kv][kbs[ci]][:], pt[p_][ci][:],
                                    start=(ci == 0), stop=(ci == nkb - 1))
                                if ci % 2 == 0:
                                    next(prev_wo, None)
                            # denominator bcast: all-ones stationary matmuls
                            # accumulate column sums over both partial accs
                            bc = psp.tile([P, 512], F32, tag="ps")
                            parts = []
                            parts.append((onesmat, accd[p_])
                                         if accd[p_] is not None else
                                         (onesmat_b, pt[p_][odds[0]]))
                            parts.append((onesmat, accg[p_])
                                         if accg[p_] is not None else
                                         (onesmat_b, pt[p_][evens[0]]))
                            for j, (om, acc) in enumerate(parts):
                                nc.tensor.matmul(bc[:], om[:], acc[:],
                                                 start=(j == 0),
                                                 stop=(j == len(parts) - 1))
                            rec = zevpool.tile([P, 512], F32, tag="rec")
                            nc.vector.reciprocal_approx_fast(rec[:], bc[:])
                            zev = zsbp.tile([P, 512], BF16, tag="zev")
                            nc.vector.tensor_mul(zev[:], pz[:], rec[:])
                            zrows.append(zev)
                    # drain leftover previous-super Wo, then arm this super's
                    for _ in prev_wo:
                        pass
                    prev_wo = wo_ops(zrows, t)
            for _ in prev_wo:
                pass

    nc.compile()
    return nc


def _host_tables():
    freqs = 1.0 / (THETA ** (np.arange(0, DH - 1, 2, dtype=np.float64) / DH))
    ang = np.arange(L, dtype=np.float64)[:, None] * freqs[None, :]  # (L, 64)
    cos = np.cos(ang)
    sin = np.sin(ang)
    cosT = np.empty((P, L), np.float32)
    sinT = np.empty((P, L), np.float32)
    cosT[0::2, :] = cos.T
    cosT[1::2, :] = cos.T
    sinT[0::2, :] = -sin.T
    sinT[1::2, :] = sin.T
    return cosT, sinT


def _host_masks():
    import ml_dtypes
    k = np.arange(P)[:, None]
    q = np.arange(256)[None, :]
    m = np.stack([
        (k <= q), (k <= q - 128), (k >= q + 1), (k >= q - 127),
    ]).astype(ml_dtypes.bfloat16)                     # (4, 128, 256)
    m2 = np.concatenate([m, m], axis=2)               # tile for head pairs
    return np.ascontiguousarray(m2.reshape(4 * P, 512))


def _pack_core_inputs(x, Wq, Wk, Wv, Wo, n, g):
    """Prepacked per-core inputs (bf16); long contiguous per-partition runs."""
    import ml_dtypes
    BF = ml_dtypes.bfloat16
    xT = np.ascontiguousarray(x[n].T)                      # (E, L)
    # xq[qtr*128+p, kt*512+c] = xT[kt*128+p, qtr*512+c]
    xq = xT.reshape(NKT, P, 4, 512).transpose(2, 1, 0, 3).reshape(4 * P, NKT * 512)
    # wqp[h*128+p, kt*128+c] = Wq.T[kt*128+p, g*1024+h*128+c]
    wqT = Wq[g * 1024:(g + 1) * 1024, :].T                 # (E, 1024)
    wqp = wqT.reshape(NKT, P, NH, DH).transpose(2, 1, 0, 3).reshape(NH * P, NKT * DH)
    # wkv[p, kt*512+j]: j<256 -> Wk.T slice, j>=256 -> Wv.T slice
    wkT = Wk[g * 256:(g + 1) * 256, :].T.reshape(NKT, P, 256)
    wvT = Wv[g * 256:(g + 1) * 256, :].T.reshape(NKT, P, 256)
    wkvp = np.concatenate([wkT, wvT], axis=2)              # (kt, p, 512)
    wkvp = wkvp.transpose(1, 0, 2).reshape(P, NKT * 512)
    woT = Wo[:, g * 1024:(g + 1) * 1024].T                 # (1024, E)
    return {
        "xq": np.ascontiguousarray(xq.astype(BF)),
        "wqp": np.ascontiguousarray(wqp.astype(BF)),
        "wkv": np.ascontiguousarray(wkvp.astype(BF)),
        "woT": np.ascontiguousarray(woT.astype(BF)),
    }


def _prepare_in_maps(x, Wq, Wk, Wv, Wo):
    cosT, sinT = _host_tables()
    masks = _host_masks()
    in_maps = []
    for c in range(8):
        n, g = c % 4, c // 4
        m = _pack_core_inputs(x, Wq, Wk, Wv, Wo, n, g)
        m.update({"cosT": cosT, "sinT": sinT, "masks": masks})
        in_maps.append(m)
    return in_maps


def kernel(x, Wq, Wk, Wv, Wo):
    global _NC
    x = np.asarray(x, np.float32)
    Wq = np.asarray(Wq, np.float32)
    Wk = np.asarray(Wk, np.float32)
    Wv = np.asarray(Wv, np.float32)
    Wo = np.asarray(Wo, np.float32)

    if _NC is None:
        _NC = build_nc()
    nc = _NC

    in_maps = _prepare_in_maps(x, Wq, Wk, Wv, Wo)

    from concourse.bass_utils import run_bass_kernel_spmd
    res = run_bass_kernel_spmd(nc, in_maps, list(range(8)), trace=False)
    out = np.empty((N, L, E), np.float32)
    for n_ in range(4):
        out[n_] = res.results[n_]["out"] + res.results[4 + n_]["out"]
    return out


if __name__ == "__main__":
    rng = np.random.default_rng(0)
    x = rng.standard_normal((N, L, E), dtype=np.float32)
    Wq = (rng.standard_normal((E, E), dtype=np.float32) * 0.02)
    Wk = (rng.standard_normal((E // D, E), dtype=np.float32) * 0.02)
    Wv = (rng.standard_normal((E // D, E), dtype=np.float32) * 0.02)
    Wo = (rng.standard_normal((E, E), dtype=np.float32) * 0.02)
    print(kernel(x, Wq, Wk, Wv, Wo).shape)


# revision 9
# speedup vs baseline: 1.4643x; 1.0025x over previous
"""Sliding-window causal GQA attention (RoPE) for Trainium2, 8-core SPMD.

Problem: x:(4,2048,2048), Wq:(2048,2048), Wk/Wv:(512,2048), Wo:(2048,2048)
  q = rope(x @ Wq.T) 16 heads, k/v = (x @ Wk.T / x @ Wv.T) 4 kv heads (GQA x4),
  causal sliding-window attention (W=1024), out = z @ Wo.T.

Sharding: 8 cores = 4 batches x 2 head-groups (8 q-heads / 2 kv-heads each).
Each core computes a partial output (its head-group's Wo contribution) for its
batch; host sums the two partials per batch.

Per-core kernel v4 (matmul operands bf16, f32 PSUM accumulation):
  - attention tiled at 128-query blocks x 4 heads per kv-group: every
    scores/PV matmul has free dim 512 = [4 heads x 128 queries] gathered from
    a [128, 4, 512] Q tile by a 2-D free access pattern.  A 128-query block
    overlaps at most 9 key blocks (vs 10 per 256-query super), with only 2
    masked boundary blocks -> ~10% less score/PV area and half the mask work.
  - RoPE: shuffle + cos-mul on DVE, sin-mul + add on Pool (the Pool engine is
    otherwise idle during the projection phase).
  - softmax denominator: running accumulators on DVE (odd key blocks) and
    Pool (even), then all-ones-stationary matmuls accumulate the broadcast
    column sums directly into PSUM -> reciprocal -> scale.
  - Wo fused per query block; z never leaves SBUF.  Wo matmuls of the
    previous block interleave between attention matmuls as PE filler while
    the scalar engine drains exp evictions.
"""

import math
import numpy as np

H = 16
D = 4
WINDOW = 1024
THETA = 10000.0
N, L, E = 4, 2048, 2048
P = 128
DH = E // H          # 128 head dim
NH = H // 2          # 8 q heads per core
NKV = 2              # kv heads per core
NB = L // P          # 16 key blocks
NKT = E // P         # 16 contraction tiles
SCALE = 1.0 / math.sqrt(DH)

_NC = None


def _kbs_for_block(b):
    """Key blocks overlapping the window of query block b (128 queries)."""
    return list(range(max(0, b - 8), b + 1))


def build_nc():
    from contextlib import ExitStack
    from concourse import bacc, tile, mybir

    F32 = mybir.dt.float32
    F32R = mybir.dt.float32r
    BF16 = mybir.dt.bfloat16
    EXP = mybir.ActivationFunctionType.Exp

    SHUF_SWAP = [i ^ 1 for i in range(32)]

    nc = bacc.Bacc("TRN2", target_bir_lowering=False, debug=False)
    # prepacked inputs (see _pack_core_inputs for layouts)
    xq = nc.dram_tensor("xq", [4 * P, NKT * 512], BF16, kind="ExternalInput").ap()
    wqp = nc.dram_tensor("wqp", [NH * P, NKT * DH], BF16, kind="ExternalInput").ap()
    wkv = nc.dram_tensor("wkv", [P, NKT * 512], BF16, kind="ExternalInput").ap()
    woT = nc.dram_tensor("woT", [NH * DH, E], BF16, kind="ExternalInput").ap()
    cosT = nc.dram_tensor("cosT", [P, L], F32, kind="ExternalInput").ap()
    sinT = nc.dram_tensor("sinT", [P, L], F32, kind="ExternalInput").ap()
    masks = nc.dram_tensor("masks", [2 * P, 512], BF16, kind="ExternalInput").ap()
    out = nc.dram_tensor("out", [L, E], F32, kind="ExternalOutput").ap()

    with tile.TileContext(nc) as tc, ExitStack() as stk:
        const = stk.enter_context(tc.tile_pool(name="const", bufs=1))
        onesmat_f = const.tile([P, P], F32, tag="onesmat_f")
        nc.vector.memset(onesmat_f[:], 1.0)
        onesmat = const.tile([P, P], F32R, tag="onesmat")
        nc.vector.tensor_copy(onesmat[:], onesmat_f[:])
        onesmat_b = const.tile([P, P], BF16, tag="onesmat_b")
        nc.vector.tensor_copy(onesmat_b[:], onesmat_f[:])
        # mask kinds (512 wide = 4 heads x 128 queries):
        # 0=diag (k<=q), 1=far (k>=q+1)
        mk = [const.tile([P, 512], BF16, tag=f"mk{i}", name=f"mk{i}") for i in range(2)]

        resid = stk.enter_context(tc.tile_pool(name="resid", bufs=1))
        kT = [resid.tile([P, L], BF16, tag=f"kT{i}", name=f"kT{i}") for i in range(NKV)]
        kvw = resid.tile([P, NKT * 512], BF16, tag="kvw")
        vt = [[resid.tile([P, P], BF16, tag=f"v{i}_{b}", name=f"v{i}_{b}") for b in range(NB)]
              for i in range(NKV)]
        wo = [resid.tile([P, E], BF16, tag=f"wo{h}", name=f"wo{h}") for h in range(NH)]

        def rope_evict(dest, psum, cos_sl, sin_sl, tmp_pool, n):
            # dest = psum * cos + pairswap(psum) * sin   (sin pre-signed)
            # DVE: shuffle + cos-mul; Pool: sin-mul + final add.
            tmp = tmp_pool.tile([P, 512], F32, tag="ropetmp", name="ropetmp")
            nc.vector.stream_shuffle(tmp[:, :n], psum, SHUF_SWAP)
            nc.gpsimd.tensor_mul(tmp[:, :n], tmp[:, :n], sin_sl)
            nc.vector.tensor_mul(dest, psum, cos_sl)
            nc.gpsimd.tensor_add(dest, dest, tmp[:, :n])

        osb = stk.enter_context(tc.tile_pool(name="osb", bufs=2))
        pp = stk.enter_context(tc.tile_pool(name="pp", bufs=2, space="PSUM"))
        psp = stk.enter_context(tc.tile_pool(name="ps", bufs=3, space="PSUM"))
        pzp = stk.enter_context(tc.tile_pool(name="pz", bufs=2, space="PSUM"))
        pop = stk.enter_context(tc.tile_pool(name="po", bufs=1, space="PSUM"))
        with tc.tile_pool(name="quarter", bufs=2) as qpool, \
             tc.tile_pool(name="wq", bufs=3) as wqpool, \
             tc.tile_pool(name="work", bufs=20) as work, \
             tc.tile_pool(name="qt", bufs=2) as qtpool, \
             tc.tile_pool(name="accp", bufs=4) as accp, \
             tc.tile_pool(name="zsb", bufs=6) as zsbp, \
             tc.tile_pool(name="zev", bufs=3) as zevpool, \
             tc.tile_pool(name="rtmp", bufs=2) as rtmp:

            def wo_ops(zrows, b):
                """Generator emitting the fused-Wo matmuls for query block b.

                Yields after each matmul so callers can interleave them as
                PE filler between attention matmuls.
                """
                for ec in range(4):
                    po = pop.tile([P, 512], F32, tag="po")
                    for h in range(NH):
                        kv, hh = h // 4, h % 4
                        nc.tensor.matmul(
                            po[:],
                            zrows[kv][:, hh * P:(hh + 1) * P],
                            wo[h][:, ec * 512:(ec + 1) * 512],
                            start=(h == 0), stop=(h == NH - 1),
                        )
                        yield
                    ot = osb.tile([P, 512], F32, tag="ot")
                    nc.scalar.copy(ot[:], po[:])
                    r0 = b * P
                    nc.sync.dma_start(
                        out=out[r0:r0 + P, ec * 512:(ec + 1) * 512],
                        in_=ot[:])

            # --- main loop ---
            prev_wo = iter(())   # filler generator for previous block's Wo
            for qtr in range(4):
                c0 = 512 * qtr
                xt = qpool.tile([P, NKT * 512], BF16, tag="xt")
                cos_q = qpool.tile([P, 512], F32, tag="cos", bufs=2)
                sin_q = qpool.tile([P, 512], F32, tag="sin", bufs=2)
                if qtr == 0:
                    # interleave kvw/x chunks so the K projection can start
                    # after the first pair lands.
                    for dc in range(4):
                        nc.sync.dma_start(
                            out=kvw[:, dc * 2048:(dc + 1) * 2048],
                            in_=wkv[:, dc * 2048:(dc + 1) * 2048])
                        nc.sync.dma_start(
                            out=xt[:, dc * 2048:(dc + 1) * 2048],
                            in_=xq[qtr * P:(qtr + 1) * P, dc * 2048:(dc + 1) * 2048])
                else:
                    for dc in range(4):
                        nc.sync.dma_start(
                            out=xt[:, dc * 2048:(dc + 1) * 2048],
                            in_=xq[qtr * P:(qtr + 1) * P, dc * 2048:(dc + 1) * 2048])
                nc.sync.dma_start(out=cos_q[:], in_=cosT[:, c0:c0 + 512])
                nc.sync.dma_start(out=sin_q[:], in_=sinT[:, c0:c0 + 512])
                if qtr == 0:
                    for i in range(2):
                        nc.sync.dma_start(out=mk[i][:], in_=masks[i * P:(i + 1) * P, :])

                def xtile(kt, a, b):
                    return xt[:, kt * 512 + a: kt * 512 + b]

                # K projection (+RoPE) for both kv heads
                for kv in range(NKV):
                    pk = pp.tile([P, 512], F32, tag="pp")
                    for kt in range(NKT):
                        nc.tensor.matmul(
                            pk[:],
                            kvw[:, kt * 512 + kv * DH: kt * 512 + (kv + 1) * DH],
                            xtile(kt, 0, 512),
                            start=(kt == 0), stop=(kt == NKT - 1),
                        )
                    rope_evict(kT[kv][:, c0:c0 + 512], pk[:], cos_q[:], sin_q[:], rtmp, 512)

                # V projection (both kv heads at once, natural layout)
                for lb in range(4):
                    pv = pp.tile([P, 512], F32, tag="pp")
                    for kt in range(NKT):
                        nc.tensor.matmul(
                            pv[:, :NKV * DH],
                            xtile(kt, lb * P, (lb + 1) * P),
                            kvw[:, kt * 512 + 256: kt * 512 + 512],
                            start=(kt == 0), stop=(kt == NKT - 1),
                        )
                    for kv in range(NKV):
                        nc.scalar.copy(vt[kv][4 * qtr + lb][:], pv[:, kv * DH:(kv + 1) * DH])

                # Q projection + RoPE into per-kv [P, 4, 512] tiles
                qT = [qtpool.tile([P, 4, 512], BF16, tag=f"qT{g_}",
                                  name=f"qT{g_}") for g_ in range(NKV)]
                for h in range(NH):
                    kv, hh = h // 4, h % 4
                    wq = wqpool.tile([P, NKT * DH], BF16, tag="wqh")
                    nc.sync.dma_start(out=wq[:], in_=wqp[h * P:(h + 1) * P, :])
                    pq = pp.tile([P, 512], F32, tag="pp")
                    for kt in range(NKT):
                        nc.tensor.matmul(
                            pq[:],
                            wq[:, kt * DH:(kt + 1) * DH],
                            xtile(kt, 0, 512),
                            start=(kt == 0), stop=(kt == NKT - 1),
                        )
                    rope_evict(qT[kv][:, hh, :], pq[:],
                               cos_q[:], sin_q[:], rtmp, 512)
                if qtr == 0:
                    # wo needed only from the first block's Wo (~40us in);
                    # queue after the projection-critical wq loads.
                    for h in range(NH):
                        nc.sync.dma_start(out=wo[h][:], in_=woT[h * P:(h + 1) * P, :])

                # attention (+interleaved previous-block Wo) per query block
                for bl in range(4):
                    b = 4 * qtr + bl
                    kbs = _kbs_for_block(b)
                    nkb = len(kbs)
                    odds = [ci for ci in range(nkb) if ci % 2 == 1]
                    evens = [ci for ci in range(nkb) if ci % 2 == 0]
                    zrows = []   # per kv: [dims, 4 heads x 128 q]
                    for kv in range(NKV):
                        qmov = qT[kv][:, :, bl * P:(bl + 1) * P]
                        pt = [None] * nkb
                        accd = None   # DVE accumulator (odd blocks)
                        accg = None   # Pool accumulator (even blocks)
                        for ci in range(nkb):
                            kb = kbs[ci]
                            ps = psp.tile([P, 512], F32, tag="ps")
                            nc.tensor.matmul(
                                ps[:],
                                kT[kv][:, kb * P:(kb + 1) * P],
                                qmov,
                                start=True, stop=True,
                            )
                            next(prev_wo, None)
                            ptt = work.tile([P, 512], BF16, tag="pt",
                                            name="ptt")
                            nc.scalar.activation(ptt[:], ps[:], EXP,
                                                 scale=SCALE)
                            if kb == b:
                                nc.vector.tensor_mul(ptt[:], ptt[:], mk[0][:])
                            elif kb == b - 8:
                                nc.vector.tensor_mul(ptt[:], ptt[:], mk[1][:])
                            pt[ci] = ptt
                            # running denominator partial sums
                            if ci % 2 == 1 and len(odds) > 1:
                                if ci == odds[1]:
                                    accd = accp.tile([P, 512], F32R,
                                                     tag="acc", name="accd")
                                    nc.vector.tensor_add(
                                        accd[:], pt[odds[0]][:], ptt[:])
                                elif ci > odds[1]:
                                    nc.vector.tensor_add(
                                        accd[:], accd[:], ptt[:])
                            elif ci % 2 == 0 and len(evens) > 1:
                                if ci == evens[1]:
                                    accg = accp.tile([P, 512], F32R,
                                                     tag="acc", name="accg")
                                    nc.gpsimd.tensor_add(
                                        accg[:], pt[evens[0]][:], ptt[:])
                                elif ci > evens[1]:
                                    nc.gpsimd.tensor_add(
                                        accg[:], accg[:], ptt[:])

                        pz = pzp.tile([P, 512], F32, tag="pz")
                        for ci in range(nkb):
                            nc.tensor.matmul(
                                pz[:], vt[kv][kbs[ci]][:], pt[ci][:],
                                start=(ci == 0), stop=(ci == nkb - 1))
                            if ci % 2 == 0:
                                next(prev_wo, None)
                        # denominator bcast: all-ones stationary matmuls
                        bc = psp.tile([P, 512], F32, tag="ps")
                        parts = []
                        if len(odds) == 1:
                            parts.append((onesmat_b, pt[odds[0]]))
                        elif len(odds) > 1:
                            parts.append((onesmat, accd))
                        if len(evens) == 1:
                            parts.append((onesmat_b, pt[evens[0]]))
                        elif len(evens) > 1:
                            parts.append((onesmat, accg))
                        for j, (om, acc) in enumerate(parts):
                            nc.tensor.matmul(bc[:], om[:], acc[:],
                                             start=(j == 0),
                                             stop=(j == len(parts) - 1))
                        rec = zevpool.tile([P, 512], F32, tag="rec")
                        nc.vector.reciprocal_approx_fast(rec[:], bc[:])
                        zev = zsbp.tile([P, 512], BF16, tag="zev")
                        nc.vector.tensor_mul(zev[:], pz[:], rec[:])
                        zrows.append(zev)
                    # drain leftover previous-block Wo, then arm this block's
                    for _ in prev_wo:
                        pass
                    prev_wo = wo_ops(zrows, b)
            for _ in prev_wo:
                pass

    nc.compile()
    return nc


def _host_tables():
    freqs = 1.0 / (THETA ** (np.arange(0, DH - 1, 2, dtype=np.float64) / DH))
    ang = np.arange(L, dtype=np.float64)[:, None] * freqs[None, :]  # (L, 64)
    cos = np.cos(ang)
    sin = np.sin(ang)
    cosT = np.empty((P, L), np.float32)
    sinT = np.empty((P, L), np.float32)
    cosT[0::2, :] = cos.T
    cosT[1::2, :] = cos.T
    sinT[0::2, :] = -sin.T
    sinT[1::2, :] = sin.T
    return cosT, sinT


def _host_masks():
    import ml_dtypes
    k = np.arange(P)[:, None]
    q = np.arange(P)[None, :]
    m = np.stack([(k <= q), (k >= q + 1)]).astype(ml_dtypes.bfloat16)
    m4 = np.concatenate([m] * 4, axis=2)              # tile for 4 heads
    return np.ascontiguousarray(m4.reshape(2 * P, 512))


def _pack_core_inputs(x, Wq, Wk, Wv, Wo, n, g):
    """Prepacked per-core inputs (bf16); long contiguous per-partition runs."""
    import ml_dtypes
    BF = ml_dtypes.bfloat16
    xT = np.ascontiguousarray(x[n].T)                      # (E, L)
    # xq[qtr*128+p, kt*512+c] = xT[kt*128+p, qtr*512+c]
    xq = xT.reshape(NKT, P, 4, 512).transpose(2, 1, 0, 3).reshape(4 * P, NKT * 512)
    # wqp[h*128+p, kt*128+c] = Wq.T[kt*128+p, g*1024+h*128+c]
    wqT = Wq[g * 1024:(g + 1) * 1024, :].T                 # (E, 1024)
    wqp = wqT.reshape(NKT, P, NH, DH).transpose(2, 1, 0, 3).reshape(NH * P, NKT * DH)
    # wkv[p, kt*512+j]: j<256 -> Wk.T slice, j>=256 -> Wv.T slice
    wkT = Wk[g * 256:(g + 1) * 256, :].T.reshape(NKT, P, 256)
    wvT = Wv[g * 256:(g + 1) * 256, :].T.reshape(NKT, P, 256)
    wkvp = np.concatenate([wkT, wvT], axis=2)              # (kt, p, 512)
    wkvp = wkvp.transpose(1, 0, 2).reshape(P, NKT * 512)
    woT = Wo[:, g * 1024:(g + 1) * 1024].T                 # (1024, E)
    return {
        "xq": np.ascontiguousarray(xq.astype(BF)),
        "wqp": np.ascontiguousarray(wqp.astype(BF)),
        "wkv": np.ascontiguousarray(wkvp.astype(BF)),
        "woT": np.ascontiguousarray(woT.astype(BF)),
    }


def _prepare_in_maps(x, Wq, Wk, Wv, Wo):
    cosT, sinT = _host_tables()
    masks = _host_masks()
    in_maps = []
    for c in range(8):
        n, g = c % 4, c // 4
        m = _pack_core_inputs(x, Wq, Wk, Wv, Wo, n, g)
        m.update({"cosT": cosT, "sinT": sinT, "masks": masks})
        in_maps.append(m)
    return in_maps


def kernel(x, Wq, Wk, Wv, Wo):
    global _NC
    x = np.asarray(x, np.float32)
    Wq = np.asarray(Wq, np.float32)
    Wk = np.asarray(Wk, np.float32)
    Wv = np.asarray(Wv, np.float32)
    Wo = np.asarray(Wo, np.float32)

    if _NC is None:
        _NC = build_nc()
    nc = _NC

    in_maps = _prepare_in_maps(x, Wq, Wk, Wv, Wo)

    from concourse.bass_utils import run_bass_kernel_spmd
    res = run_bass_kernel_spmd(nc, in_maps, list(range(8)), trace=False)
    out = np.empty((N, L, E), np.float32)
    for n_ in range(4):
        out[n_] = res.results[n_]["out"] + res.results[4 + n_]["out"]
    return out


if __name__ == "__main__":
    rng = np.random.default_rng(0)
    x = rng.standard_normal((N, L, E), dtype=np.float32)
    Wq = (rng.standard_normal((E, E), dtype=np.float32) * 0.02)
    Wk = (rng.standard_normal((E // D, E), dtype=np.float32) * 0.02)
    Wv = (rng.standard_normal((E // D, E), dtype=np.float32) * 0.02)
    Wo = (rng.standard_normal((E, E), dtype=np.float32) * 0.02)
    print(kernel(x, Wq, Wk, Wv, Wo).shape)


# revision 12
# speedup vs baseline: 1.5461x; 1.0559x over previous
"""Sliding-window causal GQA attention (RoPE) for Trainium2, 8-core SPMD.

Problem: x:(4,2048,2048), Wq:(2048,2048), Wk/Wv:(512,2048), Wo:(2048,2048)
  q = rope(x @ Wq.T) 16 heads, k/v = (x @ Wk.T / x @ Wv.T) 4 kv heads (GQA x4),
  causal sliding-window attention (W=1024), out = z @ Wo.T.

Sharding: 8 cores = 4 batches x 2 head-groups (8 q-heads / 2 kv-heads each).
Each core computes a partial output (its head-group's Wo contribution) for its
batch; host sums the two partials per batch.

Per-core kernel v4 (matmul operands bf16, f32 PSUM accumulation):
  - attention tiled at 128-query blocks x 4 heads per kv-group: every
    scores/PV matmul has free dim 512 = [4 heads x 128 queries] gathered from
    a [128, 4, 512] Q tile by a 2-D free access pattern.  A 128-query block
    overlaps at most 9 key blocks (vs 10 per 256-query super), with only 2
    masked boundary blocks -> ~10% less score/PV area and half the mask work.
  - RoPE: shuffle + cos-mul on DVE, sin-mul + add on Pool (the Pool engine is
    otherwise idle during the projection phase).
  - softmax denominator: running accumulators on DVE (odd key blocks) and
    Pool (even), then all-ones-stationary matmuls accumulate the broadcast
    column sums directly into PSUM -> reciprocal -> scale.
  - Wo fused per query block; z never leaves SBUF.  Wo matmuls of the
    previous block interleave between attention matmuls as PE filler while
    the scalar engine drains exp evictions.
"""

import math
import numpy as np

H = 16
D = 4
WINDOW = 1024
THETA = 10000.0
N, L, E = 4, 2048, 2048
P = 128
DH = E // H          # 128 head dim
NH = H // 2          # 8 q heads per core
NKV = 2              # kv heads per core
NB = L // P          # 16 key blocks
NKT = E // P         # 16 contraction tiles
SCALE = 1.0 / math.sqrt(DH)

_NC = None


def _kbs_for_block(b):
    """Key blocks overlapping the window of query block b (128 queries)."""
    return list(range(max(0, b - 8), b + 1))


def build_nc():
    from contextlib import ExitStack
    from concourse import bacc, tile, mybir

    F32 = mybir.dt.float32
    F32R = mybir.dt.float32r
    BF16 = mybir.dt.bfloat16
    EXP = mybir.ActivationFunctionType.Exp

    SHUF_SWAP = [i ^ 1 for i in range(32)]

    nc = bacc.Bacc("TRN2", target_bir_lowering=False, debug=False)
    # prepacked inputs (see _pack_core_inputs for layouts)
    xq = nc.dram_tensor("xq", [4 * P, NKT * 512], BF16, kind="ExternalInput").ap()
    wqp = nc.dram_tensor("wqp", [NH * P, NKT * DH], BF16, kind="ExternalInput").ap()
    wkv = nc.dram_tensor("wkv", [P, NKT * 512], BF16, kind="ExternalInput").ap()
    woT = nc.dram_tensor("woT", [NH * DH, E], BF16, kind="ExternalInput").ap()
    cosT = nc.dram_tensor("cosT", [P, L], BF16, kind="ExternalInput").ap()
    sinT = nc.dram_tensor("sinT", [P, L], BF16, kind="ExternalInput").ap()
    masks = nc.dram_tensor("masks", [2 * P, 512], BF16, kind="ExternalInput").ap()
    out = nc.dram_tensor("out", [L, E], F32, kind="ExternalOutput").ap()

    with tile.TileContext(nc) as tc, ExitStack() as stk:
        const = stk.enter_context(tc.tile_pool(name="const", bufs=1))
        onesmat_f = const.tile([P, P], F32, tag="onesmat_f")
        nc.vector.memset(onesmat_f[:], 1.0)
        onesmat_b = const.tile([P, P], BF16, tag="onesmat_b")
        nc.vector.tensor_copy(onesmat_b[:], onesmat_f[:])
        # mask kinds (512 wide = 4 heads x 128 queries):
        # 0=diag (k<=q), 1=far (k>=q+1)
        mk = [const.tile([P, 512], BF16, tag=f"mk{i}", name=f"mk{i}") for i in range(2)]

        resid = stk.enter_context(tc.tile_pool(name="resid", bufs=1))
        kT = [resid.tile([P, L], BF16, tag=f"kT{i}", name=f"kT{i}") for i in range(NKV)]
        kvw = resid.tile([P, NKT * 512], BF16, tag="kvw")
        vt = [[resid.tile([P, P], BF16, tag=f"v{i}_{b}", name=f"v{i}_{b}") for b in range(NB)]
              for i in range(NKV)]
        wo = [resid.tile([P, E], BF16, tag=f"wo{h}", name=f"wo{h}") for h in range(NH)]

        def rope_evict(dest, psum, cos_sl, sin_sl, tmp_pool, n):
            # dest = psum * cos + pairswap(psum * sin2);  sin2 is the sin
            # table pre-swapped/signed so the shuffle happens after the mul
            # (bf16->bf16, legal for StreamShuffle).  Pool: sin-mul + add;
            # DVE: cos-mul + shuffle.
            c = tmp_pool.tile([P, 512], BF16, tag="ropec", name="ropec")
            nc.scalar.copy(c[:, :n], psum)
            y = tmp_pool.tile([P, 512], BF16, tag="ropey", name="ropey")
            ysw = tmp_pool.tile([P, 512], BF16, tag="ropeysw", name="ropeysw")
            nc.gpsimd.tensor_mul(y[:, :n], c[:, :n], sin_sl)
            nc.vector.tensor_mul(dest, c[:, :n], cos_sl)
            nc.vector.stream_shuffle(ysw[:, :n], y[:, :n], SHUF_SWAP)
            nc.gpsimd.tensor_add(dest, dest, ysw[:, :n])

        osb = stk.enter_context(tc.tile_pool(name="osb", bufs=2))
        pp = stk.enter_context(tc.tile_pool(name="pp", bufs=2, space="PSUM"))
        psp = stk.enter_context(tc.tile_pool(name="ps", bufs=3, space="PSUM"))
        pzp = stk.enter_context(tc.tile_pool(name="pz", bufs=2, space="PSUM"))
        pop = stk.enter_context(tc.tile_pool(name="po", bufs=1, space="PSUM"))
        with tc.tile_pool(name="quarter", bufs=2) as qpool, \
             tc.tile_pool(name="wq", bufs=3) as wqpool, \
             tc.tile_pool(name="work", bufs=20) as work, \
             tc.tile_pool(name="qt", bufs=2) as qtpool, \
             tc.tile_pool(name="accp", bufs=4) as accp, \
             tc.tile_pool(name="zsb", bufs=6) as zsbp, \
             tc.tile_pool(name="zev", bufs=3) as zevpool, \
             tc.tile_pool(name="rtmp", bufs=2) as rtmp:

            def wo_ops(zrows, b):
                """Generator emitting the fused-Wo matmuls for query block b.

                Yields after each matmul so callers can interleave them as
                PE filler between attention matmuls.
                """
                for ec in range(4):
                    po = pop.tile([P, 512], F32, tag="po")
                    for h in range(NH):
                        kv, hh = h // 4, h % 4
                        nc.tensor.matmul(
                            po[:],
                            zrows[kv][:, hh * P:(hh + 1) * P],
                            wo[h][:, ec * 512:(ec + 1) * 512],
                            start=(h == 0), stop=(h == NH - 1),
                        )
                        yield
                    ot = osb.tile([P, 512], F32, tag="ot")
                    nc.scalar.copy(ot[:], po[:])
                    r0 = b * P
                    nc.sync.dma_start(
                        out=out[r0:r0 + P, ec * 512:(ec + 1) * 512],
                        in_=ot[:])

            # --- main loop ---
            prev_wo = iter(())   # filler generator for previous block's Wo
            for qtr in range(4):
                c0 = 512 * qtr
                xt = qpool.tile([P, NKT * 512], BF16, tag="xt")
                cos_q = qpool.tile([P, 512], BF16, tag="cos", bufs=2)
                sin_q = qpool.tile([P, 512], BF16, tag="sin", bufs=2)
                if qtr == 0:
                    # interleave kvw/x chunks so the K projection can start
                    # after the first pair lands.
                    for dc in range(4):
                        nc.sync.dma_start(
                            out=kvw[:, dc * 2048:(dc + 1) * 2048],
                            in_=wkv[:, dc * 2048:(dc + 1) * 2048])
                        nc.sync.dma_start(
                            out=xt[:, dc * 2048:(dc + 1) * 2048],
                            in_=xq[qtr * P:(qtr + 1) * P, dc * 2048:(dc + 1) * 2048])
                else:
                    for dc in range(4):
                        nc.sync.dma_start(
                            out=xt[:, dc * 2048:(dc + 1) * 2048],
                            in_=xq[qtr * P:(qtr + 1) * P, dc * 2048:(dc + 1) * 2048])
                nc.sync.dma_start(out=cos_q[:], in_=cosT[:, c0:c0 + 512])
                nc.sync.dma_start(out=sin_q[:], in_=sinT[:, c0:c0 + 512])
                if qtr == 0:
                    for i in range(2):
                        nc.sync.dma_start(out=mk[i][:], in_=masks[i * P:(i + 1) * P, :])

                def xtile(kt, a, b):
                    return xt[:, kt * 512 + a: kt * 512 + b]

                # K projection (+RoPE) for both kv heads
                for kv in range(NKV):
                    pk = pp.tile([P, 512], F32, tag="pp")
                    for kt in range(NKT):
                        nc.tensor.matmul(
                            pk[:],
                            kvw[:, kt * 512 + kv * DH: kt * 512 + (kv + 1) * DH],
                            xtile(kt, 0, 512),
                            start=(kt == 0), stop=(kt == NKT - 1),
                        )
                    rope_evict(kT[kv][:, c0:c0 + 512], pk[:], cos_q[:], sin_q[:], rtmp, 512)

                # V projection (both kv heads at once, natural layout)
                for lb in range(4):
                    pv = pp.tile([P, 512], F32, tag="pp")
                    for kt in range(NKT):
                        nc.tensor.matmul(
                            pv[:, :NKV * DH],
                            xtile(kt, lb * P, (lb + 1) * P),
                            kvw[:, kt * 512 + 256: kt * 512 + 512],
                            start=(kt == 0), stop=(kt == NKT - 1),
                        )
                    for kv in range(NKV):
                        nc.scalar.copy(vt[kv][4 * qtr + lb][:], pv[:, kv * DH:(kv + 1) * DH])

                # Q projection + RoPE into per-kv [P, 4, 512] tiles
                qT = [qtpool.tile([P, 4, 512], BF16, tag=f"qT{g_}",
                                  name=f"qT{g_}") for g_ in range(NKV)]
                for h in range(NH):
                    kv, hh = h // 4, h % 4
                    wq = wqpool.tile([P, NKT * DH], BF16, tag="wqh")
                    nc.sync.dma_start(out=wq[:], in_=wqp[h * P:(h + 1) * P, :])
                    pq = pp.tile([P, 512], F32, tag="pp")
                    for kt in range(NKT):
                        nc.tensor.matmul(
                            pq[:],
                            wq[:, kt * DH:(kt + 1) * DH],
                            xtile(kt, 0, 512),
                            start=(kt == 0), stop=(kt == NKT - 1),
                        )
                    rope_evict(qT[kv][:, hh, :], pq[:],
                               cos_q[:], sin_q[:], rtmp, 512)
                if qtr == 0:
                    # wo needed only from the first block's Wo (~40us in);
                    # queue after the projection-critical wq loads.
                    for h in range(NH):
                        nc.sync.dma_start(out=wo[h][:], in_=woT[h * P:(h + 1) * P, :])

                # attention (+interleaved previous-block Wo) per query block
                for bl in range(4):
                    b = 4 * qtr + bl
                    kbs = _kbs_for_block(b)
                    nkb = len(kbs)
                    odds = [ci for ci in range(nkb) if ci % 2 == 1]
                    evens = [ci for ci in range(nkb) if ci % 2 == 0]
                    zrows = []   # per kv: [dims, 4 heads x 128 q]
                    for kv in range(NKV):
                        qmov = qT[kv][:, :, bl * P:(bl + 1) * P]
                        pt = [None] * nkb
                        accd = None   # DVE accumulator (odd blocks)
                        accg = None   # Pool accumulator (even blocks)
                        for ci in range(nkb):
                            kb = kbs[ci]
                            ps = psp.tile([P, 512], F32, tag="ps")
                            nc.tensor.matmul(
                                ps[:],
                                kT[kv][:, kb * P:(kb + 1) * P],
                                qmov,
                                start=True, stop=True,
                            )
                            next(prev_wo, None)
                            ptt = work.tile([P, 512], BF16, tag="pt",
                                            name="ptt")
                            nc.scalar.activation(ptt[:], ps[:], EXP,
                                                 scale=SCALE)
                            if kb == b:
                                nc.vector.tensor_mul(ptt[:], ptt[:], mk[0][:])
                            elif kb == b - 8:
                                nc.vector.tensor_mul(ptt[:], ptt[:], mk[1][:])
                            pt[ci] = ptt
                            # running denominator partial sums
                            if ci % 2 == 0 and len(evens) > 1:
                                if ci == evens[1]:
                                    accd = accp.tile([P, 512], BF16,
                                                     tag="acc", name="accd")
                                    nc.vector.tensor_add(
                                        accd[:], pt[evens[0]][:], ptt[:])
                                elif ci > evens[1]:
                                    nc.vector.tensor_add(
                                        accd[:], accd[:], ptt[:])
                            elif ci % 2 == 1 and len(odds) > 1:
                                if ci == odds[1]:
                                    accg = accp.tile([P, 512], BF16,
                                                     tag="acc", name="accg")
                                    nc.gpsimd.tensor_add(
                                        accg[:], pt[odds[0]][:], ptt[:])
                                elif ci > odds[1]:
                                    nc.gpsimd.tensor_add(
                                        accg[:], accg[:], ptt[:])

                        pz = pzp.tile([P, 512], F32, tag="pz")
                        for ci in range(nkb):
                            nc.tensor.matmul(
                                pz[:], vt[kv][kbs[ci]][:], pt[ci][:],
                                start=(ci == 0), stop=(ci == nkb - 1))
                            if ci % 2 == 0:
                                next(prev_wo, None)
                        # denominator bcast: all-ones stationary matmuls
                        bc = psp.tile([P, 512], F32, tag="ps")
                        parts = []
                        if len(evens) == 1:
                            parts.append((onesmat_b, pt[evens[0]]))
                        elif len(evens) > 1:
                            parts.append((onesmat_b, accd))
                        if len(odds) == 1:
                            parts.append((onesmat_b, pt[odds[0]]))
                        elif len(odds) > 1:
                            parts.append((onesmat_b, accg))
                        for j, (om, acc) in enumerate(parts):
                            nc.tensor.matmul(bc[:], om[:], acc[:],
                                             start=(j == 0),
                                             stop=(j == len(parts) - 1))
                        rec = zevpool.tile([P, 512], F32, tag="rec")
                        nc.vector.reciprocal_approx_fast(rec[:], bc[:])
                        zev = zsbp.tile([P, 512], BF16, tag="zev")
                        nc.vector.tensor_mul(zev[:], pz[:], rec[:])
                        zrows.append(zev)
                    # drain leftover previous-block Wo, then arm this block's
                    for _ in prev_wo:
                        pass
                    prev_wo = wo_ops(zrows, b)
            for _ in prev_wo:
                pass

    nc.compile()
    return nc


def _host_tables():
    freqs = 1.0 / (THETA ** (np.arange(0, DH - 1, 2, dtype=np.float64) / DH))
    ang = np.arange(L, dtype=np.float64)[:, None] * freqs[None, :]  # (L, 64)
    cos = np.cos(ang)
    sin = np.sin(ang)
    import ml_dtypes
    cosT = np.empty((P, L), np.float32)
    sinT = np.empty((P, L), np.float32)
    cosT[0::2, :] = cos.T
    cosT[1::2, :] = cos.T
    sinT[0::2, :] = sin.T
    sinT[1::2, :] = -sin.T
    return cosT.astype(ml_dtypes.bfloat16), sinT.astype(ml_dtypes.bfloat16)


def _host_masks():
    import ml_dtypes
    k = np.arange(P)[:, None]
    q = np.arange(P)[None, :]
    m = np.stack([(k <= q), (k >= q + 1)]).astype(ml_dtypes.bfloat16)
    m4 = np.concatenate([m] * 4, axis=2)              # tile for 4 heads
    return np.ascontiguousarray(m4.reshape(2 * P, 512))


def _pack_core_inputs(x, Wq, Wk, Wv, Wo, n, g):
    """Prepacked per-core inputs (bf16); long contiguous per-partition runs."""
    import ml_dtypes
    BF = ml_dtypes.bfloat16
    xT = np.ascontiguousarray(x[n].T)                      # (E, L)
    # xq[qtr*128+p, kt*512+c] = xT[kt*128+p, qtr*512+c]
    xq = xT.reshape(NKT, P, 4, 512).transpose(2, 1, 0, 3).reshape(4 * P, NKT * 512)
    # wqp[h*128+p, kt*128+c] = Wq.T[kt*128+p, g*1024+h*128+c]
    wqT = Wq[g * 1024:(g + 1) * 1024, :].T                 # (E, 1024)
    wqp = wqT.reshape(NKT, P, NH, DH).transpose(2, 1, 0, 3).reshape(NH * P, NKT * DH)
    # wkv[p, kt*512+j]: j<256 -> Wk.T slice, j>=256 -> Wv.T slice
    wkT = Wk[g * 256:(g + 1) * 256, :].T.reshape(NKT, P, 256)
    wvT = Wv[g * 256:(g + 1) * 256, :].T.reshape(NKT, P, 256)
    wkvp = np.concatenate([wkT, wvT], axis=2)              # (kt, p, 512)
    wkvp = wkvp.transpose(1, 0, 2).reshape(P, NKT * 512)
    woT = Wo[:, g * 1024:(g + 1) * 1024].T                 # (1024, E)
    return {
        "xq": np.ascontiguousarray(xq.astype(BF)),
        "wqp": np.ascontiguousarray(wqp.astype(BF)),
        "wkv": np.ascontiguousarray(wkvp.astype(BF)),
        "woT": np.ascontiguousarray(woT.astype(BF)),
    }


def _prepare_in_maps(x, Wq, Wk, Wv, Wo):
    cosT, sinT = _host_tables()
    masks = _host_masks()
    in_maps = []
    for c in range(8):
        n, g = c % 4, c // 4
        m = _pack_core_inputs(x, Wq, Wk, Wv, Wo, n, g)
        m.update({"cosT": cosT, "sinT": sinT, "masks": masks})
        in_maps.append(m)
    return in_maps


def kernel(x, Wq, Wk, Wv, Wo):
    global _NC
    x = np.asarray(x, np.float32)
    Wq = np.asarray(Wq, np.float32)
    Wk = np.asarray(Wk, np.float32)
    Wv = np.asarray(Wv, np.float32)
    Wo = np.asarray(Wo, np.float32)

    if _NC is None:
        _NC = build_nc()
    nc = _NC

    in_maps = _prepare_in_maps(x, Wq, Wk, Wv, Wo)

    from concourse.bass_utils import run_bass_kernel_spmd
    res = run_bass_kernel_spmd(nc, in_maps, list(range(8)), trace=False)
    out = np.empty((N, L, E), np.float32)
    for n_ in range(4):
        out[n_] = res.results[n_]["out"] + res.results[4 + n_]["out"]
    return out


if __name__ == "__main__":
    rng = np.random.default_rng(0)
    x = rng.standard_normal((N, L, E), dtype=np.float32)
    Wq = (rng.standard_normal((E, E), dtype=np.float32) * 0.02)
    Wk = (rng.standard_normal((E // D, E), dtype=np.float32) * 0.02)
    Wv = (rng.standard_normal((E // D, E), dtype=np.float32) * 0.02)
    Wo = (rng.standard_normal((E, E), dtype=np.float32) * 0.02)
    print(kernel(x, Wq, Wk, Wv, Wo).shape)


# revision 13
# speedup vs baseline: 1.5483x; 1.0014x over previous
"""Sliding-window causal GQA attention (RoPE) for Trainium2, 8-core SPMD.

Problem: x:(4,2048,2048), Wq:(2048,2048), Wk/Wv:(512,2048), Wo:(2048,2048)
  q = rope(x @ Wq.T) 16 heads, k/v = (x @ Wk.T / x @ Wv.T) 4 kv heads (GQA x4),
  causal sliding-window attention (W=1024), out = z @ Wo.T.

Sharding: 8 cores = 4 batches x 2 head-groups (8 q-heads / 2 kv-heads each).
Each core computes a partial output (its head-group's Wo contribution) for its
batch; host sums the two partials per batch.

Per-core kernel v4 (matmul operands bf16, f32 PSUM accumulation):
  - attention tiled at 128-query blocks x 4 heads per kv-group: every
    scores/PV matmul has free dim 512 = [4 heads x 128 queries] gathered from
    a [128, 4, 512] Q tile by a 2-D free access pattern.  A 128-query block
    overlaps at most 9 key blocks (vs 10 per 256-query super), with only 2
    masked boundary blocks -> ~10% less score/PV area and half the mask work.
  - RoPE: shuffle + cos-mul on DVE, sin-mul + add on Pool (the Pool engine is
    otherwise idle during the projection phase).
  - softmax denominator: running accumulators on DVE (odd key blocks) and
    Pool (even), then all-ones-stationary matmuls accumulate the broadcast
    column sums directly into PSUM -> reciprocal -> scale.
  - Wo fused per query block; z never leaves SBUF.  Wo matmuls of the
    previous block interleave between attention matmuls as PE filler while
    the scalar engine drains exp evictions.
"""

import math
import numpy as np

H = 16
D = 4
WINDOW = 1024
THETA = 10000.0
N, L, E = 4, 2048, 2048
P = 128
DH = E // H          # 128 head dim
NH = H // 2          # 8 q heads per core
NKV = 2              # kv heads per core
NB = L // P          # 16 key blocks
NKT = E // P         # 16 contraction tiles
SCALE = 1.0 / math.sqrt(DH)

_NC = None


def _kbs_for_block(b):
    """Key blocks overlapping the window of query block b (128 queries)."""
    return list(range(max(0, b - 8), b + 1))


def build_nc():
    from contextlib import ExitStack
    from concourse import bacc, tile, mybir

    F32 = mybir.dt.float32
    F32R = mybir.dt.float32r
    BF16 = mybir.dt.bfloat16
    EXP = mybir.ActivationFunctionType.Exp

    SHUF_SWAP = [i ^ 1 for i in range(32)]

    nc = bacc.Bacc("TRN2", target_bir_lowering=False, debug=False)
    # prepacked inputs (see _pack_core_inputs for layouts)
    xq = nc.dram_tensor("xq", [4 * P, NKT * 512], BF16, kind="ExternalInput").ap()
    wqp = nc.dram_tensor("wqp", [NH * P, NKT * DH], BF16, kind="ExternalInput").ap()
    wkv = nc.dram_tensor("wkv", [P, NKT * 512], BF16, kind="ExternalInput").ap()
    woT = nc.dram_tensor("woT", [NH * DH, E], BF16, kind="ExternalInput").ap()
    cosT = nc.dram_tensor("cosT", [P, L], BF16, kind="ExternalInput").ap()
    sinT = nc.dram_tensor("sinT", [P, L], BF16, kind="ExternalInput").ap()
    masks = nc.dram_tensor("masks", [2 * P, 512], BF16, kind="ExternalInput").ap()
    out = nc.dram_tensor("out", [L, E], F32, kind="ExternalOutput").ap()

    with tile.TileContext(nc) as tc, ExitStack() as stk:
        const = stk.enter_context(tc.tile_pool(name="const", bufs=1))
        onesmat_f = const.tile([P, P], F32, tag="onesmat_f")
        nc.vector.memset(onesmat_f[:], 1.0)
        onesmat_b = const.tile([P, P], BF16, tag="onesmat_b")
        nc.vector.tensor_copy(onesmat_b[:], onesmat_f[:])
        # mask kinds (512 wide = 4 heads x 128 queries):
        # 0=diag (k<=q), 1=far (k>=q+1)
        mk = [const.tile([P, 512], BF16, tag=f"mk{i}", name=f"mk{i}") for i in range(2)]

        resid = stk.enter_context(tc.tile_pool(name="resid", bufs=1))
        kT = [resid.tile([P, L], BF16, tag=f"kT{i}", name=f"kT{i}") for i in range(NKV)]
        kvw = resid.tile([P, NKT * 512], BF16, tag="kvw")
        vt = [[resid.tile([P, P], BF16, tag=f"v{i}_{b}", name=f"v{i}_{b}") for b in range(NB)]
              for i in range(NKV)]
        wo = [resid.tile([P, E], BF16, tag=f"wo{h}", name=f"wo{h}") for h in range(NH)]

        def rope_evict(dest, psum, cos_sl, sin_sl, tmp_pool, n):
            # dest = psum * cos + pairswap(psum * sin2);  sin2 is the sin
            # table pre-swapped/signed so the shuffle happens after the mul
            # (bf16->bf16, legal for StreamShuffle).  Pool: sin-mul + add;
            # DVE: cos-mul + shuffle.
            c = tmp_pool.tile([P, 512], BF16, tag="ropec", name="ropec")
            nc.scalar.copy(c[:, :n], psum)
            y = tmp_pool.tile([P, 512], BF16, tag="ropey", name="ropey")
            ysw = tmp_pool.tile([P, 512], BF16, tag="ropeysw", name="ropeysw")
            nc.gpsimd.tensor_mul(y[:, :n], c[:, :n], sin_sl)
            nc.vector.tensor_mul(dest, c[:, :n], cos_sl)
            nc.vector.stream_shuffle(ysw[:, :n], y[:, :n], SHUF_SWAP)
            nc.gpsimd.tensor_add(dest, dest, ysw[:, :n])

        osb = stk.enter_context(tc.tile_pool(name="osb", bufs=2))
        pp = stk.enter_context(tc.tile_pool(name="pp", bufs=2, space="PSUM"))
        psp = stk.enter_context(tc.tile_pool(name="ps", bufs=3, space="PSUM"))
        pzp = stk.enter_context(tc.tile_pool(name="pz", bufs=2, space="PSUM"))
        pop = stk.enter_context(tc.tile_pool(name="po", bufs=1, space="PSUM"))
        with tc.tile_pool(name="quarter", bufs=2) as qpool, \
             tc.tile_pool(name="wq", bufs=3) as wqpool, \
             tc.tile_pool(name="work", bufs=20) as work, \
             tc.tile_pool(name="qt", bufs=2) as qtpool, \
             tc.tile_pool(name="accp", bufs=4) as accp, \
             tc.tile_pool(name="zsb", bufs=6) as zsbp, \
             tc.tile_pool(name="zev", bufs=3) as zevpool, \
             tc.tile_pool(name="rtmp", bufs=2) as rtmp:

            def wo_ops(zrows, b):
                """Generator emitting the fused-Wo matmuls for query block b.

                Yields after each matmul so callers can interleave them as
                PE filler between attention matmuls.
                """
                for ec in range(4):
                    po = pop.tile([P, 512], F32, tag="po")
                    for h in range(NH):
                        kv, hh = h // 4, h % 4
                        nc.tensor.matmul(
                            po[:],
                            zrows[kv][:, hh * P:(hh + 1) * P],
                            wo[h][:, ec * 512:(ec + 1) * 512],
                            start=(h == 0), stop=(h == NH - 1),
                        )
                        yield
                    ot = osb.tile([P, 512], F32, tag="ot")
                    nc.scalar.copy(ot[:], po[:])
                    r0 = b * P
                    nc.sync.dma_start(
                        out=out[r0:r0 + P, ec * 512:(ec + 1) * 512],
                        in_=ot[:])

            # --- main loop ---
            prev_wo = iter(())   # filler generator for previous block's Wo
            for qtr in range(4):
                c0 = 512 * qtr
                xt = qpool.tile([P, NKT * 512], BF16, tag="xt")
                cos_q = qpool.tile([P, 512], BF16, tag="cos", bufs=2)
                sin_q = qpool.tile([P, 512], BF16, tag="sin", bufs=2)
                if qtr == 0:
                    # interleave kvw/x chunks so the K projection can start
                    # after the first pair lands.
                    for dc in range(4):
                        nc.sync.dma_start(
                            out=kvw[:, dc * 2048:(dc + 1) * 2048],
                            in_=wkv[:, dc * 2048:(dc + 1) * 2048])
                        nc.sync.dma_start(
                            out=xt[:, dc * 2048:(dc + 1) * 2048],
                            in_=xq[qtr * P:(qtr + 1) * P, dc * 2048:(dc + 1) * 2048])
                else:
                    for dc in range(4):
                        nc.sync.dma_start(
                            out=xt[:, dc * 2048:(dc + 1) * 2048],
                            in_=xq[qtr * P:(qtr + 1) * P, dc * 2048:(dc + 1) * 2048])
                nc.sync.dma_start(out=cos_q[:], in_=cosT[:, c0:c0 + 512])
                nc.sync.dma_start(out=sin_q[:], in_=sinT[:, c0:c0 + 512])
                if qtr == 0:
                    for i in range(2):
                        nc.sync.dma_start(out=mk[i][:], in_=masks[i * P:(i + 1) * P, :])

                def xtile(kt, a, b):
                    return xt[:, kt * 512 + a: kt * 512 + b]

                # K projection (+RoPE) for both kv heads
                for kv in range(NKV):
                    pk = pp.tile([P, 512], F32, tag="pp")
                    for kt in range(NKT):
                        nc.tensor.matmul(
                            pk[:],
                            kvw[:, kt * 512 + kv * DH: kt * 512 + (kv + 1) * DH],
                            xtile(kt, 0, 512),
                            start=(kt == 0), stop=(kt == NKT - 1),
                        )
                    rope_evict(kT[kv][:, c0:c0 + 512], pk[:], cos_q[:], sin_q[:], rtmp, 512)

                # V projection (both kv heads at once, natural layout)
                for lb in range(4):
                    pv = pp.tile([P, 512], F32, tag="pp")
                    for kt in range(NKT):
                        nc.tensor.matmul(
                            pv[:, :NKV * DH],
                            xtile(kt, lb * P, (lb + 1) * P),
                            kvw[:, kt * 512 + 256: kt * 512 + 512],
                            start=(kt == 0), stop=(kt == NKT - 1),
                        )
                    for kv in range(NKV):
                        nc.scalar.copy(vt[kv][4 * qtr + lb][:], pv[:, kv * DH:(kv + 1) * DH])

                # Q projection + RoPE into per-kv [P, 4, 512] tiles
                qT = [qtpool.tile([P, 4, 512], BF16, tag=f"qT{g_}",
                                  name=f"qT{g_}") for g_ in range(NKV)]
                for h in range(NH):
                    kv, hh = h // 4, h % 4
                    wq = wqpool.tile([P, NKT * DH], BF16, tag="wqh")
                    nc.sync.dma_start(out=wq[:], in_=wqp[h * P:(h + 1) * P, :])
                    pq = pp.tile([P, 512], F32, tag="pp")
                    for kt in range(NKT):
                        nc.tensor.matmul(
                            pq[:],
                            wq[:, kt * DH:(kt + 1) * DH],
                            xtile(kt, 0, 512),
                            start=(kt == 0), stop=(kt == NKT - 1),
                        )
                    rope_evict(qT[kv][:, hh, :], pq[:],
                               cos_q[:], sin_q[:], rtmp, 512)
                if qtr == 0:
                    # wo needed only from the first block's Wo (~40us in);
                    # queue after the projection-critical wq loads.
                    for h in range(NH):
                        nc.sync.dma_start(out=wo[h][:], in_=woT[h * P:(h + 1) * P, :])

                # attention (+interleaved previous-block Wo) per query block
                for bl in range(4):
                    b = 4 * qtr + bl
                    kbs = _kbs_for_block(b)
                    nkb = len(kbs)
                    odds = [ci for ci in range(nkb) if ci % 2 == 1]
                    evens = [ci for ci in range(nkb) if ci % 2 == 0]
                    zrows = []   # per kv: [dims, 4 heads x 128 q]
                    for kv in range(NKV):
                        qmov = qT[kv][:, :, bl * P:(bl + 1) * P]
                        pt = [None] * nkb
                        accd = None   # DVE accumulator (odd blocks)
                        accg = None   # Pool accumulator (even blocks)
                        for ci in range(nkb):
                            kb = kbs[ci]
                            ps = psp.tile([P, 512], F32, tag="ps")
                            nc.tensor.matmul(
                                ps[:],
                                kT[kv][:, kb * P:(kb + 1) * P],
                                qmov,
                                start=True, stop=True,
                            )
                            next(prev_wo, None)
                            ptt = work.tile([P, 512], BF16, tag="pt",
                                            name="ptt")
                            nc.scalar.activation(ptt[:], ps[:], EXP,
                                                 scale=SCALE)
                            if kb == b:
                                nc.vector.tensor_mul(ptt[:], ptt[:], mk[0][:])
                            elif kb == b - 8:
                                nc.vector.tensor_mul(ptt[:], ptt[:], mk[1][:])
                            pt[ci] = ptt
                            # running denominator partial sums
                            if ci % 2 == 0 and len(evens) > 1:
                                if ci == evens[1]:
                                    accd = accp.tile([P, 512], BF16,
                                                     tag="acc", name="accd")
                                    nc.vector.tensor_add(
                                        accd[:], pt[evens[0]][:], ptt[:])
                                elif ci > evens[1]:
                                    nc.vector.tensor_add(
                                        accd[:], accd[:], ptt[:])
                            elif ci % 2 == 1 and len(odds) > 1:
                                if ci == odds[1]:
                                    accg = accp.tile([P, 512], BF16,
                                                     tag="acc", name="accg")
                                    nc.gpsimd.tensor_add(
                                        accg[:], pt[odds[0]][:], ptt[:])
                                elif ci > odds[1]:
                                    nc.gpsimd.tensor_add(
                                        accg[:], accg[:], ptt[:])

                        pz = pzp.tile([P, 512], F32, tag="pz")
                        for ci in range(nkb):
                            nc.tensor.matmul(
                                pz[:], vt[kv][kbs[ci]][:], pt[ci][:],
                                start=(ci == 0), stop=(ci == nkb - 1))
                            if ci % 2 == 0:
                                next(prev_wo, None)
                        # denominator bcast: all-ones stationary matmuls
                        bc = pp.tile([P, 512], F32, tag="pp")
                        parts = []
                        if len(evens) == 1:
                            parts.append((onesmat_b, pt[evens[0]]))
                        elif len(evens) > 1:
                            parts.append((onesmat_b, accd))
                        if len(odds) == 1:
                            parts.append((onesmat_b, pt[odds[0]]))
                        elif len(odds) > 1:
                            parts.append((onesmat_b, accg))
                        for j, (om, acc) in enumerate(parts):
                            nc.tensor.matmul(bc[:], om[:], acc[:],
                                             start=(j == 0),
                                             stop=(j == len(parts) - 1))
                        rec = zevpool.tile([P, 512], F32, tag="rec")
                        nc.vector.reciprocal_approx_fast(rec[:], bc[:])
                        zev = zsbp.tile([P, 512], BF16, tag="zev")
                        nc.vector.tensor_mul(zev[:], pz[:], rec[:])
                        zrows.append(zev)
                    # drain leftover previous-block Wo, then arm this block's
                    for _ in prev_wo:
                        pass
                    prev_wo = wo_ops(zrows, b)
            for _ in prev_wo:
                pass

    nc.compile()
    return nc


def _host_tables():
    freqs = 1.0 / (THETA ** (np.arange(0, DH - 1, 2, dtype=np.float64) / DH))
    ang = np.arange(L, dtype=np.float64)[:, None] * freqs[None, :]  # (L, 64)
    cos = np.cos(ang)
    sin = np.sin(ang)
    import ml_dtypes
    cosT = np.empty((P, L), np.float32)
    sinT = np.empty((P, L), np.float32)
    cosT[0::2, :] = cos.T
    cosT[1::2, :] = cos.T
    sinT[0::2, :] = sin.T
    sinT[1::2, :] = -sin.T
    return cosT.astype(ml_dtypes.bfloat16), sinT.astype(ml_dtypes.bfloat16)


def _host_masks():
    import ml_dtypes
    k = np.arange(P)[:, None]
    q = np.arange(P)[None, :]
    m = np.stack([(k <= q), (k >= q + 1)]).astype(ml_dtypes.bfloat16)
    m4 = np.concatenate([m] * 4, axis=2)              # tile for 4 heads
    return np.ascontiguousarray(m4.reshape(2 * P, 512))


def _pack_core_inputs(x, Wq, Wk, Wv, Wo, n, g):
    """Prepacked per-core inputs (bf16); long contiguous per-partition runs."""
    import ml_dtypes
    BF = ml_dtypes.bfloat16
    xT = np.ascontiguousarray(x[n].T)                      # (E, L)
    # xq[qtr*128+p, kt*512+c] = xT[kt*128+p, qtr*512+c]
    xq = xT.reshape(NKT, P, 4, 512).transpose(2, 1, 0, 3).reshape(4 * P, NKT * 512)
    # wqp[h*128+p, kt*128+c] = Wq.T[kt*128+p, g*1024+h*128+c]
    wqT = Wq[g * 1024:(g + 1) * 1024, :].T                 # (E, 1024)
    wqp = wqT.reshape(NKT, P, NH, DH).transpose(2, 1, 0, 3).reshape(NH * P, NKT * DH)
    # wkv[p, kt*512+j]: j<256 -> Wk.T slice, j>=256 -> Wv.T slice
    wkT = Wk[g * 256:(g + 1) * 256, :].T.reshape(NKT, P, 256)
    wvT = Wv[g * 256:(g + 1) * 256, :].T.reshape(NKT, P, 256)
    wkvp = np.concatenate([wkT, wvT], axis=2)              # (kt, p, 512)
    wkvp = wkvp.transpose(1, 0, 2).reshape(P, NKT * 512)
    woT = Wo[:, g * 1024:(g + 1) * 1024].T                 # (1024, E)
    return {
        "xq": np.ascontiguousarray(xq.astype(BF)),
        "wqp": np.ascontiguousarray(wqp.astype(BF)),
        "wkv": np.ascontiguousarray(wkvp.astype(BF)),
        "woT": np.ascontiguousarray(woT.astype(BF)),
    }


def _prepare_in_maps(x, Wq, Wk, Wv, Wo):
    cosT, sinT = _host_tables()
    masks = _host_masks()
    in_maps = []
    for c in range(8):
        n, g = c % 4, c // 4
        m = _pack_core_inputs(x, Wq, Wk, Wv, Wo, n, g)
        m.update({"cosT": cosT, "sinT": sinT, "masks": masks})
        in_maps.append(m)
    return in_maps


def kernel(x, Wq, Wk, Wv, Wo):
    global _NC
    x = np.asarray(x, np.float32)
    Wq = np.asarray(Wq, np.float32)
    Wk = np.asarray(Wk, np.float32)
    Wv = np.asarray(Wv, np.float32)
    Wo = np.asarray(Wo, np.float32)

    if _NC is None:
        _NC = build_nc()
    nc = _NC

    in_maps = _prepare_in_maps(x, Wq, Wk, Wv, Wo)

    from concourse.bass_utils import run_bass_kernel_spmd
    res = run_bass_kernel_spmd(nc, in_maps, list(range(8)), trace=False)
    out = np.empty((N, L, E), np.float32)
    for n_ in range(4):
        out[n_] = res.results[n_]["out"] + res.results[4 + n_]["out"]
    return out


if __name__ == "__main__":
    rng = np.random.default_rng(0)
    x = rng.standard_normal((N, L, E), dtype=np.float32)
    Wq = (rng.standard_normal((E, E), dtype=np.float32) * 0.02)
    Wk = (rng.standard_normal((E // D, E), dtype=np.float32) * 0.02)
    Wv = (rng.standard_normal((E // D, E), dtype=np.float32) * 0.02)
    Wo = (rng.standard_normal((E, E), dtype=np.float32) * 0.02)
    print(kernel(x, Wq, Wk, Wv, Wo).shape)
